# revision 28
# baseline (speedup 1.0000x reference)
"""Trainium2 Bass kernel for CausalSelfAttention (B=4, T=2048, C=768, H=6, D=128)
with RoPE + QK-RMSNorm.

Sharding: 8 cores = batch(4) x head-group(2, 3 heads each). Host sums the two
head-group c_proj partials per batch.

v3 design:
  - bf16 data plane (inputs, K/Q/V/A tiles): halves input DMA, enables DVE
    2x elementwise modes, removes the f32r 4-cycle penalty on narrow matmuls.
  - K's RMS-norm is folded into the exp's per-partition scale AP (never
    applied to the K tile).
  - softmax denominator comes free from the AV matmul: AV is computed
    transposed (out [q,129]) with a ones column appended to V, so column 128
    accumulates sum_k(A); normalized z^T transposes back via PE transposes.
  - phase A computes ALL projections + RoPE + norms (Q tiles persistent), so
    phase B's Activation engine runs Exp only -- no act-table reloads.
  - phase B interleaves qt's AV/tails/c_proj with qt+1's S/exp stream.
"""

import numpy as np

_B, _T, _C, _H, _D = 4, 2048, 768, 6, 128
_HPG = 3            # heads per group
_HD = _HPG * _D     # 384, per-group head dims
_NT = 4             # T tiles of 512
_TW = 512           # tile width (T_q)
_NKC = _T // 128    # 16 k-chunks of 128
_NCB = _C // 128    # 6 c_in chunks
_EPS = 1e-15

_cached = {}


def _build_nc():
    from contextlib import ExitStack
    from concourse import bacc, tile, mybir

    f32 = mybir.dt.float32
    f32r = mybir.dt.float32r
    bf16 = mybir.dt.bfloat16
    Act = mybir.ActivationFunctionType

    nc = bacc.Bacc("TRN2", target_bir_lowering=False, debug=False)

    xT = nc.dram_tensor("xT", (_C, _T), bf16, kind="ExternalInput").ap()
    wq = nc.dram_tensor("wq", (_C, _HD), bf16, kind="ExternalInput").ap()
    wk = nc.dram_tensor("wk", (_C, _HD), bf16, kind="ExternalInput").ap()
    wv = nc.dram_tensor("wv", (_C, _HD), bf16, kind="ExternalInput").ap()
    wo = nc.dram_tensor("wo", (_HD, _C), bf16, kind="ExternalInput").ap()
    cc = nc.dram_tensor("cc", (128, _T), bf16, kind="ExternalInput").ap()
    ss = nc.dram_tensor("ss", (128, _T), bf16, kind="ExternalInput").ap()
    # cst: [tri(128) | ident(128) | ones_col(1)] in bf16
    cst = nc.dram_tensor("cst", (128, 257), bf16, kind="ExternalInput").ap()
    perm = nc.dram_tensor("perm", (128, 128), bf16, kind="ExternalInput").ap()
    onesr = nc.dram_tensor("onesr", (1, 128), bf16, kind="ExternalInput").ap()
    out = nc.dram_tensor("out", (_T, _C), bf16, kind="ExternalOutput").ap()

    with tile.TileContext(nc) as tc, ExitStack() as ctx, \
            nc.allow_low_precision(reason="bf16 data plane; psum accumulate f32"):
        # --- pools ---
        pc = ctx.enter_context(tc.tile_pool(name="pc", bufs=1))
        pg = ctx.enter_context(tc.tile_pool(name="pg", bufs=3))        # scratch
        pa = ctx.enter_context(tc.tile_pool(name="pa", bufs=1))        # A tiles
        psm = ctx.enter_context(tc.tile_pool(name="psm", bufs=3))      # small rows
        # psum pools (8 banks total)
        pps = ctx.enter_context(tc.tile_pool(name="pps", bufs=3, space="PSUM"))
        ppo = ctx.enter_context(tc.tile_pool(name="ppo", bufs=2, space="PSUM"))
        ppq = ctx.enter_context(tc.tile_pool(name="ppq", bufs=2, space="PSUM"))
        ppd = ctx.enter_context(tc.tile_pool(name="ppd", bufs=1, space="PSUM"))

        # --- inputs resident in SBUF (load order: first-needed first) ---
        t_wk, t_xt, t_wv, t_wq = [], [], [], []
        for c in range(_NCB):
            t = pc.tile([128, _HD], bf16, tag=f"wk{c}", name=f"wk{c}")
            nc.sync.dma_start(t[:], wk[c * 128:(c + 1) * 128, :])
            t_wk.append(t)
        for c in range(_NCB):
            t = pc.tile([128, _T], bf16, tag=f"xt{c}", name=f"xt{c}")
            nc.sync.dma_start(t[:], xT[c * 128:(c + 1) * 128, :])
            t_xt.append(t)
        for c in range(_NCB):
            t = pc.tile([128, _HD], bf16, tag=f"wv{c}", name=f"wv{c}")
            nc.sync.dma_start(t[:], wv[c * 128:(c + 1) * 128, :])
            t_wv.append(t)
        for c in range(_NCB):
            t = pc.tile([128, _HD], bf16, tag=f"wq{c}", name=f"wq{c}")
            nc.sync.dma_start(t[:], wq[c * 128:(c + 1) * 128, :])
            t_wq.append(t)
        t_cc = pc.tile([128, _T], bf16, tag="cc")
        t_ss = pc.tile([128, _T], bf16, tag="ss")
        nc.sync.dma_start(t_cc[:], cc[:])
        nc.sync.dma_start(t_ss[:], ss[:])
        t_cst = pc.tile([128, 257], bf16, tag="cst")
        t_perm = pc.tile([128, 128], bf16, tag="perm")
        t_onesr = pc.tile([1, 128], bf16, tag="onesr")
        nc.sync.dma_start(t_cst[:], cst[:])
        nc.sync.dma_start(t_perm[:], perm[:])
        nc.sync.dma_start(t_onesr[:], onesr[:])
        t_wo = []
        for c in range(_HPG):
            t = pc.tile([128, _C], bf16, tag=f"wo{c}", name=f"wo{c}")
            nc.sync.dma_start(t[:], wo[c * 128:(c + 1) * 128, :])
            t_wo.append(t)

        t_tri = t_cst[:, 0:128]
        t_ident = t_cst[:, 128:256]
        t_ones_col = t_cst[:, 256:257]

        t_epsA = pc.tile([128, 1], f32, tag="epsA")   # K: 128*eps
        nc.gpsimd.memset(t_epsA[:], 128.0 * _EPS)
        t_epsB = pc.tile([128, 1], f32, tag="epsB")   # Q: eps
        nc.gpsimd.memset(t_epsB[:], _EPS)

        # persistent K^T (rope'd, UN-normalized) per head; V blocks w/ ones col
        t_kn = [pc.tile([128, _T], bf16, tag=f"kn{h}", name=f"kn{h}")
                for h in range(_HPG)]
        t_vo = [pc.tile([128, 387], bf16, tag=f"vo{tb}", name=f"vo{tb}")
                for tb in range(_NKC)]
        # exp scale columns: c_k = 1/sqrt(ms_k + 128 eps); col h*16+kc
        t_ck = pc.tile([128, _HPG * _NKC], f32, tag="ck", name="t_ck")
        # all Q tiles (rope'd + normalized), persistent through phase B
        t_q = [[pc.tile([128, _TW], bf16, tag=f"q{qt}_{h}", name=f"q{qt}_{h}")
                for h in range(_HPG)] for qt in range(_NT)]
        # A tiles: 16 k-chunks x 3 heads, reused across qt
        t_a = [[pa.tile([128, _TW], bf16, tag=f"a{h}_{kc}", name=f"a{h}_{kc}")
                for kc in range(_NKC)] for h in range(_HPG)]
        t_zT = [pc.tile([128, _TW], bf16, tag=f"zT{h}", name=f"zT{h}")
                for h in range(_HPG)]
        t_z = [pc.tile([128, _TW], bf16, tag=f"z{h}", name=f"z{h}")
               for h in range(_HPG)]

        def rope(dst_ap, col0):
            """In-place RoPE on dst_ap (128, 512) bf16 sbuf tile slice.
            cc-mul on Pool (sbuf-only), psum-mul + add on DVE."""
            csl = slice(col0, col0 + _TW)
            p_sw = ppq.tile([128, _TW], f32, tag="pq", name="p_sw")
            nc.tensor.matmul(p_sw[:], t_perm[:], dst_ap, start=True, stop=True)
            t_sw = pg.tile([128, _TW], bf16, tag="sw", name="t_sw")
            nc.gpsimd.tensor_mul(dst_ap, dst_ap, t_cc[:, csl])
            nc.vector.tensor_mul(t_sw[:], p_sw[:], t_ss[:, csl])
            nc.vector.tensor_add(dst_ap, dst_ap, t_sw[:])

        # ============ Phase A: projections, RoPE, norms ============
        for i in range(_NT):
            isl = slice(i * _TW, (i + 1) * _TW)
            for h in range(_HPG):
                hsl = slice(h * 128, (h + 1) * 128)
                p_k = pps.tile([128, _TW], f32, tag="ps", name="p_k")
                for c in range(_NCB):
                    nc.tensor.matmul(p_k[:], t_wk[c][:, hsl], t_xt[c][:, isl],
                                     start=(c == 0), stop=(c == _NCB - 1))
                nc.scalar.copy(t_kn[h][:, isl], p_k[:])
        def v_proj(tb0, tb1):
            for tb in range(tb0, tb1):
                bsl = slice(tb * 128, (tb + 1) * 128)
                p_v = ppo.tile([128, _HD], f32, tag="po", name="p_v")
                for c in range(_NCB):
                    nc.tensor.matmul(p_v[:], t_xt[c][:, bsl], t_wv[c][:],
                                     start=(c == 0), stop=(c == _NCB - 1))
                vo_v = t_vo[tb][:].rearrange("p (h d) -> p h d", h=_HPG)[:, :, 0:128]
                pv_v = p_v[:].rearrange("p (h d) -> p h d", h=_HPG)
                nc.scalar.copy(vo_v, pv_v)
                nc.gpsimd.memset(t_vo[tb][:].rearrange("p (h d) -> p h d", h=_HPG)
                                 [:, :, 128:129], 1.0)
        v_proj(0, 8)
        # K rope + norm stats per T-tile i; c_k col layout: i*12 + h*4 + j.
        # rsqrt via exp(-0.5 ln(x)) -- Ln and Exp share an act table, so the
        # Activation engine never reloads tables mid-kernel.
        def k_phase(i):
            isl = slice(i * _TW, (i + 1) * _TW)
            msl = slice(i * 12, (i + 1) * 12)
            for h in range(_HPG):
                rope(t_kn[h][:, isl], i * _TW)
            p_msk = ppd.tile([128, 12], f32, tag="pd", name="p_msk")
            for h in range(_HPG):
                t_sq = pg.tile([128, _TW], bf16, tag="sq", name="t_sq")
                nc.vector.tensor_mul(t_sq[:], t_kn[h][:, isl], t_kn[h][:, isl])
                for j in range(4):
                    col = h * 4 + j
                    nc.tensor.matmul(p_msk[:, col:col + 1],
                                     t_sq[:, j * 128:(j + 1) * 128],
                                     t_ones_col, start=True, stop=True)
            t_lk = pg.tile([128, 12], f32, tag="lq", name="t_lk")
            nc.scalar.activation(t_lk[:], p_msk[:], Act.Ln,
                                 bias=t_epsA[:], scale=1.0)
            nc.scalar.activation(t_ck[:, msl], t_lk[:], Act.Exp, scale=-0.5)

        def q_phase(qt):
            """Q projections + rope + RMS-norm for one T_q tile (3 heads)."""
            qsl = slice(qt * _TW, (qt + 1) * _TW)
            for h in range(_HPG):
                hsl = slice(h * 128, (h + 1) * 128)
                p_q = ppq.tile([128, _TW], f32, tag="pq", name="p_q")
                for c in range(_NCB):
                    nc.tensor.matmul(p_q[:], t_wq[c][:, hsl], t_xt[c][:, qsl],
                                     start=(c == 0), stop=(c == _NCB - 1))
                nc.vector.tensor_copy(t_q[qt][h][:], p_q[:])
            for h in range(_HPG):
                rope(t_q[qt][h][:], qt * _TW)
            p_msq = ppd.tile([128, 12], f32, tag="pd", name="p_msq")
            for h in range(_HPG):
                t_sq = pg.tile([128, _TW], bf16, tag="sq", name="t_sq")
                nc.vector.tensor_mul(t_sq[:], t_q[qt][h][:], t_q[qt][h][:])
                for qb in range(4):
                    col = h * 4 + qb
                    nc.tensor.matmul(p_msq[:, col:col + 1],
                                     t_sq[:, qb * 128:(qb + 1) * 128],
                                     t_ones_col, start=True, stop=True)
            t_lq = pg.tile([128, 12], f32, tag="lq", name="t_lq")
            nc.scalar.activation(t_lq[:], p_msq[:], Act.Ln,
                                 bias=t_epsB[:], scale=1.0 / 128.0)
            t_cq12 = pg.tile([128, 12], bf16, tag="cq12", name="t_cq12",
                             bufs=2)
            nc.scalar.activation(t_cq12[:], t_lq[:], Act.Exp, scale=-0.5)
            for h in range(_HPG):
                p_rq = ppd.tile([1, _TW], bf16, tag="pd", name="p_rq",
                                padded_shape=[1, 2 * _TW])
                for qb in range(4):
                    col = h * 4 + qb
                    nc.tensor.transpose(p_rq[0:1, qb * 128:(qb + 1) * 128],
                                        t_cq12[:, col:col + 1], t_ident)
                t_rqr = psm.tile([1, _TW], bf16, tag="rqr", name="t_rqr")
                nc.vector.tensor_copy(t_rqr[:], p_rq[:])
                p_bc = ppd.tile([128, _TW], f32, tag="pd", name="p_bc")
                nc.tensor.matmul(p_bc[:], t_onesr[:], t_rqr[:],
                                 start=True, stop=True)
                nc.vector.tensor_mul(t_q[qt][h][:], t_q[qt][h][:], p_bc[:])

        # ============ Phase B: attention + c_proj, qt-pipelined ============
        def emit_s_exp(qt):
            nchunk = 4 * qt + 4
            for h in range(_HPG):
                for kc in range(nchunk):
                    roff = 0 if kc < 4 * qt else (kc - 4 * qt) * 128
                    nsl = slice(roff, _TW)
                    ksl = slice(kc * 128, (kc + 1) * 128)
                    p_s = pps.tile([128, _TW], f32, tag="ps", name="p_s")
                    nc.tensor.matmul(p_s[:, nsl], t_kn[h][:, ksl],
                                     t_q[qt][h][:, nsl], start=True, stop=True)
                    ckc = (kc // 4) * 12 + h * 4 + (kc % 4)
                    nc.scalar.activation(t_a[h][kc][:, nsl], p_s[:, nsl],
                                         Act.Exp,
                                         scale=t_ck[:, ckc:ckc + 1])
                    if kc >= 4 * qt:  # diagonal chunk: triangular mask
                        dsl = slice(roff, roff + 128)
                        nc.gpsimd.tensor_mul(t_a[h][kc][:, dsl],
                                             t_a[h][kc][:, dsl], t_tri)

        def emit_av(qt):
            for h in range(_HPG):
                vsl = slice(h * 129, (h + 1) * 129)
                for qb in range(4):
                    qbsl = slice(qb * 128, (qb + 1) * 128)
                    kmax = 4 * qt + qb
                    p_ot = ppo.tile([128, 129], f32, tag="po", name="p_ot")
                    for kc in range(kmax + 1):
                        nc.tensor.matmul(p_ot[:], t_a[h][kc][:, qbsl],
                                         t_vo[kc][:, vsl],
                                         start=(kc == 0), stop=(kc == kmax))
                    t_rd = psm.tile([128, 1], f32, tag="rd", name="t_rd")
                    nc.vector.reciprocal(t_rd[:], p_ot[:, 128:129])
                    nc.vector.tensor_scalar_mul(t_zT[h][:, qbsl],
                                                p_ot[:, 0:128], t_rd[:])
                p_z2 = pps.tile([128, _TW], bf16, tag="ps", name="p_z2",
                                padded_shape=[128, 2 * _TW])
                for qb in range(4):
                    qbsl = slice(qb * 128, (qb + 1) * 128)
                    nc.tensor.transpose(p_z2[:, qbsl], t_zT[h][:, qbsl],
                                        t_ident)
                nc.vector.tensor_copy(t_z[h][:], p_z2[:])

        def emit_tail_cproj(qt):
            for tb in range(4):
                bsl = slice(tb * 128, (tb + 1) * 128)
                rsl = slice(qt * _TW + tb * 128, qt * _TW + (tb + 1) * 128)
                t_ob = pg.tile([128, _C], bf16, tag="ob", name="t_ob", bufs=2)
                for nh in range(2):
                    osl = slice(nh * 384, (nh + 1) * 384)
                    p_c = ppq.tile([128, 384], f32, tag="pq", name="p_c")
                    for c in range(_HPG):
                        nc.tensor.matmul(p_c[:], t_z[c][:, bsl],
                                         t_wo[c][:, osl],
                                         start=(c == 0), stop=(c == _HPG - 1))
                    if nh == 0:
                        nc.scalar.copy(t_ob[:, osl], p_c[:])
                    else:
                        nc.vector.tensor_copy(t_ob[:, osl], p_c[:])
                nc.sync.dma_start(out[rsl, :], t_ob[:])

        k_phase(0)
        q_phase(0)
        emit_s_exp(0)
        q_phase(1)
        for qt in range(_NT):
            if qt == 1:
                v_proj(8, 12)
            elif qt == 2:
                v_proj(12, 16)
            emit_av(qt)
            if qt + 1 < _NT:
                k_phase(qt + 1)
                emit_s_exp(qt + 1)
            if qt + 2 < _NT:
                q_phase(qt + 2)
            emit_tail_cproj(qt)

    nc.compile()
    return nc


def _get_nc():
    if "nc" not in _cached:
        _cached["nc"] = _build_nc()
    return _cached["nc"]


def make_in_maps(x, cos, sin, Wq, Wk, Wv, Wo):
    import ml_dtypes
    bf = ml_dtypes.bfloat16

    cosT = np.ascontiguousarray(cos.reshape(_T, _D // 2).T)  # (64, T)
    sinT = np.ascontiguousarray(sin.reshape(_T, _D // 2).T)
    cc = np.concatenate([cosT, cosT], axis=0)                # (128, T)
    ss = np.concatenate([sinT, -sinT], axis=0)
    tri = (np.arange(128)[None, :] >= np.arange(128)[:, None]).astype(np.float32)
    ident = np.eye(128, dtype=np.float32)
    cst = np.concatenate([tri, ident, np.ones((128, 1), np.float32)], axis=1)
    permm = np.zeros((128, 128), dtype=np.float32)           # half-swap perm
    for d in range(64):
        permm[64 + d, d] = 1.0
        permm[d, 64 + d] = 1.0
    onesr = np.ones((1, 128), dtype=np.float32)
    in_maps = []
    for core in range(8):
        b, g = divmod(core, 2)
        gsl = slice(g * _HD, (g + 1) * _HD)
        in_maps.append({
            "xT": np.ascontiguousarray(x[b].T).astype(bf),
            "wq": np.ascontiguousarray(Wq[gsl, :].T).astype(bf),
            "wk": np.ascontiguousarray(Wk[gsl, :].T).astype(bf),
            "wv": np.ascontiguousarray(Wv[gsl, :].T).astype(bf),
            "wo": np.ascontiguousarray(Wo[:, gsl].T).astype(bf),
            "cc": cc.astype(bf), "ss": ss.astype(bf),
            "cst": cst.astype(bf), "perm": permm.astype(bf),
            "onesr": onesr.astype(bf),
        })
    return in_maps


def kernel(x, cos, sin, Wq, Wk, Wv, Wo):
    from concourse.bass_utils import run_bass_kernel_spmd

    x = np.asarray(x, dtype=np.float32)
    cos = np.asarray(cos, dtype=np.float32)
    sin = np.asarray(sin, dtype=np.float32)
    Wq = np.asarray(Wq, dtype=np.float32)
    Wk = np.asarray(Wk, dtype=np.float32)
    Wv = np.asarray(Wv, dtype=np.float32)
    Wo = np.asarray(Wo, dtype=np.float32)

    nc = _get_nc()
    in_maps = make_in_maps(x, cos, sin, Wq, Wk, Wv, Wo)
    res = run_bass_kernel_spmd(nc, in_maps, core_ids=list(range(8)))
    outs = [np.asarray(r_["out"], dtype=np.float32) for r_ in res.results]
    return np.stack([outs[2 * b] + outs[2 * b + 1] for b in range(_B)], axis=0)


# revision 30
# speedup vs baseline: 1.0117x; 1.0117x over previous
"""Trainium2 Bass kernel for CausalSelfAttention (B=4, T=2048, C=768, H=6, D=128)
with RoPE + QK-RMSNorm.

Sharding: 8 cores = batch(4) x head-group(2, 3 heads each). Host sums the two
head-group c_proj partials per batch.

v3 design:
  - bf16 data plane (inputs, K/Q/V/A tiles): halves input DMA, enables DVE
    2x elementwise modes, removes the f32r 4-cycle penalty on narrow matmuls.
  - K's RMS-norm is folded into the exp's per-partition scale AP (never
    applied to the K tile).
  - softmax denominator comes free from the AV matmul: AV is computed
    transposed (out [q,129]) with a ones column appended to V, so column 128
    accumulates sum_k(A); normalized z^T transposes back via PE transposes.
  - phase A computes ALL projections + RoPE + norms (Q tiles persistent), so
    phase B's Activation engine runs Exp only -- no act-table reloads.
  - phase B interleaves qt's AV/tails/c_proj with qt+1's S/exp stream.
"""

import numpy as np

_B, _T, _C, _H, _D = 4, 2048, 768, 6, 128
_HPG = 3            # heads per group
_HD = _HPG * _D     # 384, per-group head dims
_NT = 4             # T tiles of 512
_TW = 512           # tile width (T_q)
_NKC = _T // 128    # 16 k-chunks of 128
_NCB = _C // 128    # 6 c_in chunks
_EPS = 1e-15

_cached = {}


def _build_nc():
    from contextlib import ExitStack
    from concourse import bacc, tile, mybir

    f32 = mybir.dt.float32
    f32r = mybir.dt.float32r
    bf16 = mybir.dt.bfloat16
    Act = mybir.ActivationFunctionType

    nc = bacc.Bacc("TRN2", target_bir_lowering=False, debug=False)
    _cached.setdefault("sections", []).clear()

    def _mark(label):
        _cached["sections"].append((label, nc.get_next_instruction_name()))

    xT = nc.dram_tensor("xT", (_C, _T), bf16, kind="ExternalInput").ap()
    wq = nc.dram_tensor("wq", (_C, _HD), bf16, kind="ExternalInput").ap()
    wk = nc.dram_tensor("wk", (_C, _HD), bf16, kind="ExternalInput").ap()
    wv = nc.dram_tensor("wv", (_C, _HD), bf16, kind="ExternalInput").ap()
    wo = nc.dram_tensor("wo", (_HD, _C), bf16, kind="ExternalInput").ap()
    cc = nc.dram_tensor("cc", (128, _T), bf16, kind="ExternalInput").ap()
    ss = nc.dram_tensor("ss", (128, _T), bf16, kind="ExternalInput").ap()
    # cst: [tri(128) | ident(128) | ones_col(1)] in bf16
    cst = nc.dram_tensor("cst", (128, 257), bf16, kind="ExternalInput").ap()
    perm = nc.dram_tensor("perm", (128, 128), bf16, kind="ExternalInput").ap()
    onesr = nc.dram_tensor("onesr", (1, 128), bf16, kind="ExternalInput").ap()
    out = nc.dram_tensor("out", (_T, _C), bf16, kind="ExternalOutput").ap()

    with tile.TileContext(nc) as tc, ExitStack() as ctx, \
            nc.allow_low_precision(reason="bf16 data plane; psum accumulate f32"):
        # --- pools ---
        pc = ctx.enter_context(tc.tile_pool(name="pc", bufs=1))
        pg = ctx.enter_context(tc.tile_pool(name="pg", bufs=3))        # scratch
        pa = ctx.enter_context(tc.tile_pool(name="pa", bufs=1))        # A tiles
        psm = ctx.enter_context(tc.tile_pool(name="psm", bufs=3))      # small rows
        # psum pools (8 banks total)
        pps = ctx.enter_context(tc.tile_pool(name="pps", bufs=3, space="PSUM"))
        ppo = ctx.enter_context(tc.tile_pool(name="ppo", bufs=2, space="PSUM"))
        ppq = ctx.enter_context(tc.tile_pool(name="ppq", bufs=2, space="PSUM"))
        ppd = ctx.enter_context(tc.tile_pool(name="ppd", bufs=1, space="PSUM"))

        # --- inputs resident in SBUF (load order: first-needed first) ---
        t_wk, t_xt, t_wv, t_wq = [], [], [], []
        for c in range(_NCB):
            tw = pc.tile([128, _HD], bf16, tag=f"wk{c}", name=f"wk{c}")
            nc.sync.dma_start(tw[:], wk[c * 128:(c + 1) * 128, :])
            t_wk.append(tw)
            tx = pc.tile([128, _T], bf16, tag=f"xt{c}", name=f"xt{c}")
            nc.sync.dma_start(tx[:], xT[c * 128:(c + 1) * 128, :])
            t_xt.append(tx)
        for c in range(_NCB):
            t = pc.tile([128, _HD], bf16, tag=f"wv{c}", name=f"wv{c}")
            nc.sync.dma_start(t[:], wv[c * 128:(c + 1) * 128, :])
            t_wv.append(t)
        for c in range(_NCB):
            t = pc.tile([128, _HD], bf16, tag=f"wq{c}", name=f"wq{c}")
            nc.sync.dma_start(t[:], wq[c * 128:(c + 1) * 128, :])
            t_wq.append(t)
        t_cc = pc.tile([128, _T], bf16, tag="cc")
        t_ss = pc.tile([128, _T], bf16, tag="ss")
        nc.sync.dma_start(t_cc[:], cc[:])
        nc.sync.dma_start(t_ss[:], ss[:])
        t_cst = pc.tile([128, 257], bf16, tag="cst")
        t_perm = pc.tile([128, 128], bf16, tag="perm")
        t_onesr = pc.tile([1, 128], bf16, tag="onesr")
        nc.sync.dma_start(t_cst[:], cst[:])
        nc.sync.dma_start(t_perm[:], perm[:])
        nc.sync.dma_start(t_onesr[:], onesr[:])
        t_wo = []
        for c in range(_HPG):
            t = pc.tile([128, _C], bf16, tag=f"wo{c}", name=f"wo{c}")
            nc.sync.dma_start(t[:], wo[c * 128:(c + 1) * 128, :])
            t_wo.append(t)

        t_tri = t_cst[:, 0:128]
        t_ident = t_cst[:, 128:256]
        t_ones_col = t_cst[:, 256:257]

        t_epsA = pc.tile([128, 1], f32, tag="epsA")   # K: 128*eps
        nc.gpsimd.memset(t_epsA[:], 128.0 * _EPS)
        t_epsB = pc.tile([128, 1], f32, tag="epsB")   # Q: eps
        nc.gpsimd.memset(t_epsB[:], _EPS)

        # persistent K^T (rope'd, UN-normalized) per head; V blocks w/ ones col
        t_kn = [pc.tile([128, _T], bf16, tag=f"kn{h}", name=f"kn{h}")
                for h in range(_HPG)]
        t_vo = [pc.tile([128, 387], bf16, tag=f"vo{tb}", name=f"vo{tb}")
                for tb in range(_NKC)]
        # exp scale columns: c_k = 1/sqrt(ms_k + 128 eps); col h*16+kc
        t_ck = pc.tile([128, _HPG * _NKC], f32, tag="ck", name="t_ck")
        # all Q tiles (rope'd + normalized), persistent through phase B
        t_q = [[pc.tile([128, _TW], bf16, tag=f"q{qt}_{h}", name=f"q{qt}_{h}")
                for h in range(_HPG)] for qt in range(_NT)]
        # A tiles: 16 k-chunks x 3 heads, reused across qt
        t_a = [[pa.tile([128, _TW], bf16, tag=f"a{h}_{kc}", name=f"a{h}_{kc}")
                for kc in range(_NKC)] for h in range(_HPG)]
        t_zT = [pc.tile([128, _TW], bf16, tag=f"zT{h}", name=f"zT{h}")
                for h in range(_HPG)]
        t_z = [pc.tile([128, _TW], bf16, tag=f"z{h}", name=f"z{h}")
               for h in range(_HPG)]

        def rope(dst_ap, col0):
            """In-place RoPE on dst_ap (128, 512) bf16 sbuf tile slice.
            cc-mul on Pool (sbuf-only), psum-mul + add on DVE."""
            csl = slice(col0, col0 + _TW)
            p_sw = pps.tile([128, _TW], f32, tag="ps", name="p_sw")
            nc.tensor.matmul(p_sw[:], t_perm[:], dst_ap, start=True, stop=True)
            t_sw = pg.tile([128, _TW], bf16, tag="sw", name="t_sw")
            nc.gpsimd.tensor_mul(dst_ap, dst_ap, t_cc[:, csl])
            nc.vector.tensor_mul(t_sw[:], p_sw[:], t_ss[:, csl])
            nc.vector.tensor_add(dst_ap, dst_ap, t_sw[:])

        # ============ Phase A: projections, RoPE, norms ============
        for i in range(_NT):
            isl = slice(i * _TW, (i + 1) * _TW)
            for h in range(_HPG):
                hsl = slice(h * 128, (h + 1) * 128)
                p_k = pps.tile([128, _TW], f32, tag="ps", name="p_k")
                for c in range(_NCB):
                    nc.tensor.matmul(p_k[:], t_wk[c][:, hsl], t_xt[c][:, isl],
                                     start=(c == 0), stop=(c == _NCB - 1))
                nc.scalar.copy(t_kn[h][:, isl], p_k[:])
        def v_proj(tb0, tb1):
            _mark(f'vproj{tb0}')
            for tb in range(tb0, tb1):
                bsl = slice(tb * 128, (tb + 1) * 128)
                p_v = ppo.tile([128, _HD], f32, tag="po", name="p_v")
                for c in range(_NCB):
                    nc.tensor.matmul(p_v[:], t_xt[c][:, bsl], t_wv[c][:],
                                     start=(c == 0), stop=(c == _NCB - 1))
                vo_v = t_vo[tb][:].rearrange("p (h d) -> p h d", h=_HPG)[:, :, 0:128]
                pv_v = p_v[:].rearrange("p (h d) -> p h d", h=_HPG)
                nc.scalar.copy(vo_v, pv_v)
                nc.gpsimd.memset(t_vo[tb][:].rearrange("p (h d) -> p h d", h=_HPG)
                                 [:, :, 128:129], 1.0)
        v_proj(0, 8)
        # K rope + norm stats per T-tile i; c_k col layout: i*12 + h*4 + j.
        # rsqrt via exp(-0.5 ln(x)) -- Ln and Exp share an act table, so the
        # Activation engine never reloads tables mid-kernel.
        def k_phase(i):
            _mark(f'k_phase{i}')
            isl = slice(i * _TW, (i + 1) * _TW)
            msl = slice(i * 12, (i + 1) * 12)
            for h in range(_HPG):
                rope(t_kn[h][:, isl], i * _TW)
            p_msk = ppd.tile([128, 12], f32, tag="pd", name="p_msk")
            for h in range(_HPG):
                t_sq = pg.tile([128, _TW], bf16, tag="sq", name="t_sq")
                nc.vector.tensor_mul(t_sq[:], t_kn[h][:, isl], t_kn[h][:, isl])
                for j in range(4):
                    col = h * 4 + j
                    nc.tensor.matmul(p_msk[:, col:col + 1],
                                     t_sq[:, j * 128:(j + 1) * 128],
                                     t_ones_col, start=True, stop=True)
            t_lk = pg.tile([128, 12], f32, tag="lq", name="t_lk")
            nc.scalar.activation(t_lk[:], p_msk[:], Act.Ln,
                                 bias=t_epsA[:], scale=1.0)
            nc.scalar.activation(t_ck[:, msl], t_lk[:], Act.Exp, scale=-0.5)

        def q_phase(qt):
            _mark(f'q_phase{qt}')
            """Q projections + rope + RMS-norm for one T_q tile (3 heads)."""
            qsl = slice(qt * _TW, (qt + 1) * _TW)
            for h in range(_HPG):
                hsl = slice(h * 128, (h + 1) * 128)
                p_q = ppq.tile([128, _TW], f32, tag="pq", name="p_q")
                for c in range(_NCB):
                    nc.tensor.matmul(p_q[:], t_wq[c][:, hsl], t_xt[c][:, qsl],
                                     start=(c == 0), stop=(c == _NCB - 1))
                nc.vector.tensor_copy(t_q[qt][h][:], p_q[:])
            for h in range(_HPG):
                rope(t_q[qt][h][:], qt * _TW)
            p_msq = ppd.tile([128, 12], f32, tag="pd", name="p_msq")
            for h in range(_HPG):
                t_sq = pg.tile([128, _TW], bf16, tag="sq", name="t_sq")
                nc.vector.tensor_mul(t_sq[:], t_q[qt][h][:], t_q[qt][h][:])
                for qb in range(4):
                    col = h * 4 + qb
                    nc.tensor.matmul(p_msq[:, col:col + 1],
                                     t_sq[:, qb * 128:(qb + 1) * 128],
                                     t_ones_col, start=True, stop=True)
            t_lq = pg.tile([128, 12], f32, tag="lq", name="t_lq")
            nc.scalar.activation(t_lq[:], p_msq[:], Act.Ln,
                                 bias=t_epsB[:], scale=1.0 / 128.0)
            t_cq12 = pg.tile([128, 12], bf16, tag="cq12", name="t_cq12",
                             bufs=2)
            nc.scalar.activation(t_cq12[:], t_lq[:], Act.Exp, scale=-0.5)
            for h in range(_HPG):
                p_rq = ppd.tile([1, _TW], bf16, tag="pd", name="p_rq",
                                padded_shape=[1, 2 * _TW])
                for qb in range(4):
                    col = h * 4 + qb
                    nc.tensor.transpose(p_rq[0:1, qb * 128:(qb + 1) * 128],
                                        t_cq12[:, col:col + 1], t_ident)
                t_rqr = psm.tile([1, _TW], bf16, tag="rqr", name="t_rqr")
                nc.vector.tensor_copy(t_rqr[:], p_rq[:])
                p_bc = ppd.tile([128, _TW], f32, tag="pd", name="p_bc")
                nc.tensor.matmul(p_bc[:], t_onesr[:], t_rqr[:],
                                 start=True, stop=True)
                nc.vector.tensor_mul(t_q[qt][h][:], t_q[qt][h][:], p_bc[:])

        # ============ Phase B: attention + c_proj, qt-pipelined ============
        def emit_s_exp(qt, fillers=()):
            _mark(f's_exp{qt}')
            fillers = list(fillers)
            nchunk = 4 * qt + 4
            for h in range(_HPG):
                if fillers:
                    fillers.pop(0)()
                for kc in range(nchunk):
                    roff = 0 if kc < 4 * qt else (kc - 4 * qt) * 128
                    nsl = slice(roff, _TW)
                    ksl = slice(kc * 128, (kc + 1) * 128)
                    p_s = pps.tile([128, _TW], f32, tag="ps", name="p_s")
                    nc.tensor.matmul(p_s[:, nsl], t_kn[h][:, ksl],
                                     t_q[qt][h][:, nsl], start=True, stop=True)
                    ckc = (kc // 4) * 12 + h * 4 + (kc % 4)
                    nc.scalar.activation(t_a[h][kc][:, nsl], p_s[:, nsl],
                                         Act.Exp,
                                         scale=t_ck[:, ckc:ckc + 1])
                    if kc >= 4 * qt:  # diagonal chunk: triangular mask
                        dsl = slice(roff, roff + 128)
                        nc.gpsimd.tensor_mul(t_a[h][kc][:, dsl],
                                             t_a[h][kc][:, dsl], t_tri)

        def emit_av(qt):
            _mark(f'av{qt}')
            for h in range(_HPG):
                vsl = slice(h * 129, (h + 1) * 129)
                for qb in range(4):
                    qbsl = slice(qb * 128, (qb + 1) * 128)
                    kmax = 4 * qt + qb
                    p_ot = ppo.tile([128, 129], f32, tag="po", name="p_ot")
                    for kc in range(kmax + 1):
                        nc.tensor.matmul(p_ot[:], t_a[h][kc][:, qbsl],
                                         t_vo[kc][:, vsl],
                                         start=(kc == 0), stop=(kc == kmax))
                    t_rd = psm.tile([128, 1], f32, tag="rd", name="t_rd")
                    nc.vector.reciprocal(t_rd[:], p_ot[:, 128:129])
                    nc.vector.tensor_scalar_mul(t_zT[h][:, qbsl],
                                                p_ot[:, 0:128], t_rd[:])
                p_z2 = pps.tile([128, _TW], bf16, tag="ps", name="p_z2",
                                padded_shape=[128, 2 * _TW])
                for qb in range(4):
                    qbsl = slice(qb * 128, (qb + 1) * 128)
                    nc.tensor.transpose(p_z2[:, qbsl], t_zT[h][:, qbsl],
                                        t_ident)
                nc.scalar.copy(t_z[h][:], p_z2[:])

        def emit_cproj_tb(qt, tbs):
            _mark(f'cproj{qt}')
            for tb in tbs:
                bsl = slice(tb * 128, (tb + 1) * 128)
                rsl = slice(qt * _TW + tb * 128, qt * _TW + (tb + 1) * 128)
                t_ob = pg.tile([128, _C], bf16, tag="ob", name="t_ob", bufs=2)
                for nh in range(2):
                    osl = slice(nh * 384, (nh + 1) * 384)
                    p_c = ppq.tile([128, 384], f32, tag="pq", name="p_c")
                    for c in range(_HPG):
                        nc.tensor.matmul(p_c[:], t_z[c][:, bsl],
                                         t_wo[c][:, osl],
                                         start=(c == 0), stop=(c == _HPG - 1))
                    if nh == 0:
                        nc.scalar.copy(t_ob[:, osl], p_c[:])
                    else:
                        nc.vector.tensor_copy(t_ob[:, osl], p_c[:])
                nc.sync.dma_start(out[rsl, :], t_ob[:])

        k_phase(0)
        q_phase(0)
        emit_s_exp(0)
        q_phase(1)
        for qt in range(_NT):
            if qt == 1:
                v_proj(8, 12)
            elif qt == 2:
                v_proj(12, 16)
            emit_av(qt)
            if qt + 1 < _NT:
                k_phase(qt + 1)
                emit_s_exp(qt + 1, fillers=[
                    lambda: emit_cproj_tb(qt, (0, 1)),
                    lambda: emit_cproj_tb(qt, (2, 3)),
                ])
            else:
                emit_cproj_tb(qt, (0, 1, 2, 3))
            if qt + 2 < _NT:
                q_phase(qt + 2)

    nc.compile()
    return nc


def _get_nc():
    if "nc" not in _cached:
        _cached["nc"] = _build_nc()
    return _cached["nc"]


def make_in_maps(x, cos, sin, Wq, Wk, Wv, Wo):
    import ml_dtypes
    bf = ml_dtypes.bfloat16

    cosT = np.ascontiguousarray(cos.reshape(_T, _D // 2).T)  # (64, T)
    sinT = np.ascontiguousarray(sin.reshape(_T, _D // 2).T)
    cc = np.concatenate([cosT, cosT], axis=0)                # (128, T)
    ss = np.concatenate([sinT, -sinT], axis=0)
    tri = (np.arange(128)[None, :] >= np.arange(128)[:, None]).astype(np.float32)
    ident = np.eye(128, dtype=np.float32)
    cst = np.concatenate([tri, ident, np.ones((128, 1), np.float32)], axis=1)
    permm = np.zeros((128, 128), dtype=np.float32)           # half-swap perm
    for d in range(64):
        permm[64 + d, d] = 1.0
        permm[d, 64 + d] = 1.0
    onesr = np.ones((1, 128), dtype=np.float32)
    in_maps = []
    for core in range(8):
        b, g = divmod(core, 2)
        gsl = slice(g * _HD, (g + 1) * _HD)
        in_maps.append({
            "xT": np.ascontiguousarray(x[b].T).astype(bf),
            "wq": np.ascontiguousarray(Wq[gsl, :].T).astype(bf),
            "wk": np.ascontiguousarray(Wk[gsl, :].T).astype(bf),
            "wv": np.ascontiguousarray(Wv[gsl, :].T).astype(bf),
            "wo": np.ascontiguousarray(Wo[:, gsl].T).astype(bf),
            "cc": cc.astype(bf), "ss": ss.astype(bf),
            "cst": cst.astype(bf), "perm": permm.astype(bf),
            "onesr": onesr.astype(bf),
        })
    return in_maps


def kernel(x, cos, sin, Wq, Wk, Wv, Wo):
    from concourse.bass_utils import run_bass_kernel_spmd

    x = np.asarray(x, dtype=np.float32)
    cos = np.asarray(cos, dtype=np.float32)
    sin = np.asarray(sin, dtype=np.float32)
    Wq = np.asarray(Wq, dtype=np.float32)
    Wk = np.asarray(Wk, dtype=np.float32)
    Wv = np.asarray(Wv, dtype=np.float32)
    Wo = np.asarray(Wo, dtype=np.float32)

    nc = _get_nc()
    in_maps = make_in_maps(x, cos, sin, Wq, Wk, Wv, Wo)
    res = run_bass_kernel_spmd(nc, in_maps, core_ids=list(range(8)))
    outs = [np.asarray(r_["out"], dtype=np.float32) for r_ in res.results]
    return np.stack([outs[2 * b] + outs[2 * b + 1] for b in range(_B)], axis=0)


# revision 31
# speedup vs baseline: 1.0791x; 1.0666x over previous
"""Trainium2 Bass kernel for CausalSelfAttention (B=4, T=2048, C=768, H=6, D=128)
with RoPE + QK-RMSNorm.

Sharding: 8 cores = batch(4) x head-group(2, 3 heads each). Host sums the two
head-group c_proj partials per batch.

v3 design:
  - bf16 data plane (inputs, K/Q/V/A tiles): halves input DMA, enables DVE
    2x elementwise modes, removes the f32r 4-cycle penalty on narrow matmuls.
  - K's RMS-norm is folded into the exp's per-partition scale AP (never
    applied to the K tile).
  - softmax denominator comes free from the AV matmul: AV is computed
    transposed (out [q,129]) with a ones column appended to V, so column 128
    accumulates sum_k(A); normalized z^T transposes back via PE transposes.
  - phase A computes ALL projections + RoPE + norms (Q tiles persistent), so
    phase B's Activation engine runs Exp only -- no act-table reloads.
  - phase B interleaves qt's AV/tails/c_proj with qt+1's S/exp stream.
"""

import numpy as np

_B, _T, _C, _H, _D = 4, 2048, 768, 6, 128
_HPG = 3            # heads per group
_HD = _HPG * _D     # 384, per-group head dims
_NT = 4             # T tiles of 512
_TW = 512           # tile width (T_q)
_NKC = _T // 128    # 16 k-chunks of 128
_NCB = _C // 128    # 6 c_in chunks
_EPS = 1e-15

_cached = {}


def _build_nc():
    from contextlib import ExitStack
    from concourse import bacc, tile, mybir

    f32 = mybir.dt.float32
    f32r = mybir.dt.float32r
    bf16 = mybir.dt.bfloat16
    Act = mybir.ActivationFunctionType

    nc = bacc.Bacc("TRN2", target_bir_lowering=False, debug=False)
    _cached.setdefault("sections", []).clear()

    def _mark(label):
        _cached["sections"].append((label, nc.get_next_instruction_name()))

    xT = nc.dram_tensor("xT", (_C, _T), bf16, kind="ExternalInput").ap()
    wq = nc.dram_tensor("wq", (_C, _HD), bf16, kind="ExternalInput").ap()
    wk = nc.dram_tensor("wk", (_C, _HD), bf16, kind="ExternalInput").ap()
    wv = nc.dram_tensor("wv", (_C, _HD), bf16, kind="ExternalInput").ap()
    wo = nc.dram_tensor("wo", (_HD, _C), bf16, kind="ExternalInput").ap()
    cc = nc.dram_tensor("cc", (128, _T), bf16, kind="ExternalInput").ap()
    ss = nc.dram_tensor("ss", (128, _T), bf16, kind="ExternalInput").ap()
    # cst: [tri(128) | ident(128) | ones_col(1)] in bf16
    cst = nc.dram_tensor("cst", (128, 257), bf16, kind="ExternalInput").ap()
    perm = nc.dram_tensor("perm", (128, 128), bf16, kind="ExternalInput").ap()
    onesr = nc.dram_tensor("onesr", (1, 128), bf16, kind="ExternalInput").ap()
    out = nc.dram_tensor("out", (_T, _C), bf16, kind="ExternalOutput").ap()

    with tile.TileContext(nc) as tc, ExitStack() as ctx, \
            nc.allow_low_precision(reason="bf16 data plane; psum accumulate f32"):
        # --- pools ---
        pc = ctx.enter_context(tc.tile_pool(name="pc", bufs=1))
        pg = ctx.enter_context(tc.tile_pool(name="pg", bufs=3))        # scratch
        pa = ctx.enter_context(tc.tile_pool(name="pa", bufs=1))        # A tiles
        psm = ctx.enter_context(tc.tile_pool(name="psm", bufs=3))      # small rows
        # psum pools (8 banks total)
        pps = ctx.enter_context(tc.tile_pool(name="pps", bufs=3, space="PSUM"))
        ppo = ctx.enter_context(tc.tile_pool(name="ppo", bufs=2, space="PSUM"))
        ppq = ctx.enter_context(tc.tile_pool(name="ppq", bufs=2, space="PSUM"))
        ppd = ctx.enter_context(tc.tile_pool(name="ppd", bufs=1, space="PSUM"))

        # --- inputs resident in SBUF (load order: first-needed first) ---
        t_wk, t_xt, t_wv, t_wq = [], [], [], []
        for c in range(_NCB):
            tw = pc.tile([128, _HD], bf16, tag=f"wk{c}", name=f"wk{c}")
            nc.sync.dma_start(tw[:], wk[c * 128:(c + 1) * 128, :])
            t_wk.append(tw)
            tx = pc.tile([128, _T], bf16, tag=f"xt{c}", name=f"xt{c}")
            nc.sync.dma_start(tx[:], xT[c * 128:(c + 1) * 128, :])
            t_xt.append(tx)
        for c in range(_NCB):
            t = pc.tile([128, _HD], bf16, tag=f"wv{c}", name=f"wv{c}")
            nc.sync.dma_start(t[:], wv[c * 128:(c + 1) * 128, :])
            t_wv.append(t)
        for c in range(_NCB):
            t = pc.tile([128, _HD], bf16, tag=f"wq{c}", name=f"wq{c}")
            nc.sync.dma_start(t[:], wq[c * 128:(c + 1) * 128, :])
            t_wq.append(t)
        t_cc = pc.tile([128, _T], bf16, tag="cc")
        t_ss = pc.tile([128, _T], bf16, tag="ss")
        nc.sync.dma_start(t_cc[:], cc[:])
        nc.sync.dma_start(t_ss[:], ss[:])
        t_cst = pc.tile([128, 257], bf16, tag="cst")
        t_perm = pc.tile([128, 128], bf16, tag="perm")
        t_onesr = pc.tile([1, 128], bf16, tag="onesr")
        nc.sync.dma_start(t_cst[:], cst[:])
        nc.sync.dma_start(t_perm[:], perm[:])
        nc.sync.dma_start(t_onesr[:], onesr[:])
        t_wo = []
        for c in range(_HPG):
            t = pc.tile([128, _C], bf16, tag=f"wo{c}", name=f"wo{c}")
            nc.sync.dma_start(t[:], wo[c * 128:(c + 1) * 128, :])
            t_wo.append(t)

        t_tri = t_cst[:, 0:128]
        t_ident = t_cst[:, 128:256]
        t_ones_col = t_cst[:, 256:257]

        t_epsA = pc.tile([128, 1], f32, tag="epsA")   # K: 128*eps
        nc.gpsimd.memset(t_epsA[:], 128.0 * _EPS)
        t_epsB = pc.tile([128, 1], f32, tag="epsB")   # Q: eps
        nc.gpsimd.memset(t_epsB[:], _EPS)

        # persistent K^T (rope'd, UN-normalized) per head; V blocks w/ ones col
        t_kn = [pc.tile([128, _T], bf16, tag=f"kn{h}", name=f"kn{h}")
                for h in range(_HPG)]
        t_vo = [pc.tile([128, 387], bf16, tag=f"vo{tb}", name=f"vo{tb}")
                for tb in range(_NKC)]
        # exp scale columns: c_k = 1/sqrt(ms_k + 128 eps); col h*16+kc
        t_ck = pc.tile([128, _HPG * _NKC], f32, tag="ck", name="t_ck")
        # all Q tiles (rope'd + normalized), persistent through phase B
        t_q = [[pc.tile([128, _TW], bf16, tag=f"q{qt}_{h}", name=f"q{qt}_{h}")
                for h in range(_HPG)] for qt in range(_NT)]
        # A tiles: 16 k-chunks x 3 heads, reused across qt
        t_a = [[pa.tile([128, _TW], bf16, tag=f"a{h}_{kc}", name=f"a{h}_{kc}")
                for kc in range(_NKC)] for h in range(_HPG)]
        t_zT = [pc.tile([128, _TW], bf16, tag=f"zT{h}", name=f"zT{h}")
                for h in range(_HPG)]
        t_z = [pc.tile([128, _TW], bf16, tag=f"z{h}", name=f"z{h}")
               for h in range(_HPG)]

        def rope(dst_ap, col0):
            """In-place RoPE on dst_ap (128, 512) bf16 sbuf tile slice.
            Half-swap via SBUF->SBUF DMA; cc-mul on Pool, rest on DVE (2x)."""
            csl = slice(col0, col0 + _TW)
            t_sw = pg.tile([128, _TW], bf16, tag="sw", name="t_sw")
            nc.sync.dma_start(t_sw[0:64, :], dst_ap[64:128, :])
            nc.sync.dma_start(t_sw[64:128, :], dst_ap[0:64, :])
            nc.gpsimd.tensor_mul(dst_ap, dst_ap, t_cc[:, csl])
            nc.vector.tensor_mul(t_sw[:], t_sw[:], t_ss[:, csl])
            nc.vector.tensor_add(dst_ap, dst_ap, t_sw[:])

        # ============ Phase A: projections, RoPE, norms ============
        for i in range(_NT):
            isl = slice(i * _TW, (i + 1) * _TW)
            for h in range(_HPG):
                hsl = slice(h * 128, (h + 1) * 128)
                p_k = pps.tile([128, _TW], f32, tag="ps", name="p_k")
                for c in range(_NCB):
                    nc.tensor.matmul(p_k[:], t_wk[c][:, hsl], t_xt[c][:, isl],
                                     start=(c == 0), stop=(c == _NCB - 1))
                nc.vector.tensor_copy(t_kn[h][:, isl], p_k[:])
        def v_proj(tb0, tb1):
            _mark(f'vproj{tb0}')
            for tb in range(tb0, tb1):
                bsl = slice(tb * 128, (tb + 1) * 128)
                p_v = ppo.tile([128, _HD], f32, tag="po", name="p_v")
                for c in range(_NCB):
                    nc.tensor.matmul(p_v[:], t_xt[c][:, bsl], t_wv[c][:],
                                     start=(c == 0), stop=(c == _NCB - 1))
                vo_v = t_vo[tb][:].rearrange("p (h d) -> p h d", h=_HPG)[:, :, 0:128]
                pv_v = p_v[:].rearrange("p (h d) -> p h d", h=_HPG)
                nc.vector.tensor_copy(vo_v, pv_v)
                nc.gpsimd.memset(t_vo[tb][:].rearrange("p (h d) -> p h d", h=_HPG)
                                 [:, :, 128:129], 1.0)
        v_proj(0, 8)
        # K rope + norm stats per T-tile i; c_k col layout: i*12 + h*4 + j.
        # rsqrt via exp(-0.5 ln(x)) -- Ln and Exp share an act table, so the
        # Activation engine never reloads tables mid-kernel.
        def k_phase(i):
            _mark(f'k_phase{i}')
            isl = slice(i * _TW, (i + 1) * _TW)
            msl = slice(i * 12, (i + 1) * 12)
            for h in range(_HPG):
                rope(t_kn[h][:, isl], i * _TW)
            p_msk = ppd.tile([128, 12], f32, tag="pd", name="p_msk")
            for h in range(_HPG):
                t_sq = pg.tile([128, _TW], bf16, tag="sq", name="t_sq")
                nc.vector.tensor_mul(t_sq[:], t_kn[h][:, isl], t_kn[h][:, isl])
                for j in range(4):
                    col = h * 4 + j
                    nc.tensor.matmul(p_msk[:, col:col + 1],
                                     t_sq[:, j * 128:(j + 1) * 128],
                                     t_ones_col, start=True, stop=True)
            t_lk = pg.tile([128, 12], f32, tag="lq", name="t_lk")
            nc.scalar.activation(t_lk[:], p_msk[:], Act.Ln,
                                 bias=t_epsA[:], scale=1.0)
            nc.scalar.activation(t_ck[:, msl], t_lk[:], Act.Exp, scale=-0.5)

        def q_phase(qt):
            _mark(f'q_phase{qt}')
            """Q projections + rope + RMS-norm for one T_q tile (3 heads)."""
            qsl = slice(qt * _TW, (qt + 1) * _TW)
            for h in range(_HPG):
                hsl = slice(h * 128, (h + 1) * 128)
                p_q = ppq.tile([128, _TW], f32, tag="pq", name="p_q")
                for c in range(_NCB):
                    nc.tensor.matmul(p_q[:], t_wq[c][:, hsl], t_xt[c][:, qsl],
                                     start=(c == 0), stop=(c == _NCB - 1))
                nc.vector.tensor_copy(t_q[qt][h][:], p_q[:])
            for h in range(_HPG):
                rope(t_q[qt][h][:], qt * _TW)
            p_msq = ppd.tile([128, 12], f32, tag="pd", name="p_msq")
            for h in range(_HPG):
                t_sq = pg.tile([128, _TW], bf16, tag="sq", name="t_sq")
                nc.vector.tensor_mul(t_sq[:], t_q[qt][h][:], t_q[qt][h][:])
                for qb in range(4):
                    col = h * 4 + qb
                    nc.tensor.matmul(p_msq[:, col:col + 1],
                                     t_sq[:, qb * 128:(qb + 1) * 128],
                                     t_ones_col, start=True, stop=True)
            t_lq = pg.tile([128, 12], f32, tag="lq", name="t_lq")
            nc.scalar.activation(t_lq[:], p_msq[:], Act.Ln,
                                 bias=t_epsB[:], scale=1.0 / 128.0)
            t_cq12 = pg.tile([128, 12], bf16, tag="cq12", name="t_cq12",
                             bufs=2)
            nc.scalar.activation(t_cq12[:], t_lq[:], Act.Exp, scale=-0.5)
            for h in range(_HPG):
                p_rq = ppd.tile([1, _TW], bf16, tag="pd", name="p_rq",
                                padded_shape=[1, 2 * _TW])
                for qb in range(4):
                    col = h * 4 + qb
                    nc.tensor.transpose(p_rq[0:1, qb * 128:(qb + 1) * 128],
                                        t_cq12[:, col:col + 1], t_ident)
                t_rqr = psm.tile([1, _TW], bf16, tag="rqr", name="t_rqr")
                nc.vector.tensor_copy(t_rqr[:], p_rq[:])
                p_bc = ppd.tile([128, _TW], f32, tag="pd", name="p_bc")
                nc.tensor.matmul(p_bc[:], t_onesr[:], t_rqr[:],
                                 start=True, stop=True)
                nc.vector.tensor_mul(t_q[qt][h][:], t_q[qt][h][:], p_bc[:])

        # ============ Phase B: attention + c_proj, qt-pipelined ============
        def emit_s_exp(qt, fillers=()):
            _mark(f's_exp{qt}')
            fillers = list(fillers)
            nchunk = 4 * qt + 4
            for h in range(_HPG):
                if fillers:
                    fillers.pop(0)()
                for kc in range(nchunk):
                    roff = 0 if kc < 4 * qt else (kc - 4 * qt) * 128
                    nsl = slice(roff, _TW)
                    ksl = slice(kc * 128, (kc + 1) * 128)
                    p_s = pps.tile([128, _TW], f32, tag="ps", name="p_s")
                    nc.tensor.matmul(p_s[:, nsl], t_kn[h][:, ksl],
                                     t_q[qt][h][:, nsl], start=True, stop=True)
                    ckc = (kc // 4) * 12 + h * 4 + (kc % 4)
                    nc.scalar.activation(t_a[h][kc][:, nsl], p_s[:, nsl],
                                         Act.Exp,
                                         scale=t_ck[:, ckc:ckc + 1])
                    if kc >= 4 * qt:  # diagonal chunk: triangular mask
                        dsl = slice(roff, roff + 128)
                        nc.gpsimd.tensor_mul(t_a[h][kc][:, dsl],
                                             t_a[h][kc][:, dsl], t_tri)

        def emit_av(qt):
            _mark(f'av{qt}')
            for h in range(_HPG):
                vsl = slice(h * 129, (h + 1) * 129)
                for qb in range(4):
                    qbsl = slice(qb * 128, (qb + 1) * 128)
                    kmax = 4 * qt + qb
                    p_ot = ppo.tile([128, 129], f32, tag="po", name="p_ot")
                    for kc in range(kmax + 1):
                        nc.tensor.matmul(p_ot[:], t_a[h][kc][:, qbsl],
                                         t_vo[kc][:, vsl],
                                         start=(kc == 0), stop=(kc == kmax))
                    t_rd = psm.tile([128, 1], f32, tag="rd", name="t_rd")
                    nc.vector.reciprocal(t_rd[:], p_ot[:, 128:129])
                    nc.vector.tensor_scalar_mul(t_zT[h][:, qbsl],
                                                p_ot[:, 0:128], t_rd[:])
                p_z2 = pps.tile([128, _TW], bf16, tag="ps", name="p_z2",
                                padded_shape=[128, 2 * _TW])
                for qb in range(4):
                    qbsl = slice(qb * 128, (qb + 1) * 128)
                    nc.tensor.transpose(p_z2[:, qbsl], t_zT[h][:, qbsl],
                                        t_ident)
                nc.scalar.copy(t_z[h][:], p_z2[:])

        def emit_cproj_tb(qt, tbs):
            _mark(f'cproj{qt}')
            for tb in tbs:
                bsl = slice(tb * 128, (tb + 1) * 128)
                rsl = slice(qt * _TW + tb * 128, qt * _TW + (tb + 1) * 128)
                t_ob = pg.tile([128, _C], bf16, tag="ob", name="t_ob", bufs=2)
                for nh in range(2):
                    osl = slice(nh * 384, (nh + 1) * 384)
                    p_c = ppq.tile([128, 384], f32, tag="pq", name="p_c")
                    for c in range(_HPG):
                        nc.tensor.matmul(p_c[:], t_z[c][:, bsl],
                                         t_wo[c][:, osl],
                                         start=(c == 0), stop=(c == _HPG - 1))
                    if nh == 0:
                        nc.scalar.copy(t_ob[:, osl], p_c[:])
                    else:
                        nc.vector.tensor_copy(t_ob[:, osl], p_c[:])
                nc.sync.dma_start(out[rsl, :], t_ob[:])

        k_phase(0)
        q_phase(0)
        emit_s_exp(0)
        q_phase(1)
        for qt in range(_NT):
            if qt == 1:
                v_proj(8, 12)
            elif qt == 2:
                v_proj(12, 16)
            emit_av(qt)
            if qt + 1 < _NT:
                k_phase(qt + 1)
                emit_s_exp(qt + 1, fillers=[
                    lambda: emit_cproj_tb(qt, (0, 1)),
                    lambda: emit_cproj_tb(qt, (2, 3)),
                ])
            else:
                emit_cproj_tb(qt, (0, 1, 2, 3))
            if qt + 2 < _NT:
                q_phase(qt + 2)

    nc.compile()
    return nc


def _get_nc():
    if "nc" not in _cached:
        _cached["nc"] = _build_nc()
    return _cached["nc"]


def make_in_maps(x, cos, sin, Wq, Wk, Wv, Wo):
    import ml_dtypes
    bf = ml_dtypes.bfloat16

    cosT = np.ascontiguousarray(cos.reshape(_T, _D // 2).T)  # (64, T)
    sinT = np.ascontiguousarray(sin.reshape(_T, _D // 2).T)
    cc = np.concatenate([cosT, cosT], axis=0)                # (128, T)
    ss = np.concatenate([sinT, -sinT], axis=0)
    tri = (np.arange(128)[None, :] >= np.arange(128)[:, None]).astype(np.float32)
    ident = np.eye(128, dtype=np.float32)
    cst = np.concatenate([tri, ident, np.ones((128, 1), np.float32)], axis=1)
    permm = np.zeros((128, 128), dtype=np.float32)           # half-swap perm
    for d in range(64):
        permm[64 + d, d] = 1.0
        permm[d, 64 + d] = 1.0
    onesr = np.ones((1, 128), dtype=np.float32)
    in_maps = []
    for core in range(8):
        b, g = divmod(core, 2)
        gsl = slice(g * _HD, (g + 1) * _HD)
        in_maps.append({
            "xT": np.ascontiguousarray(x[b].T).astype(bf),
            "wq": np.ascontiguousarray(Wq[gsl, :].T).astype(bf),
            "wk": np.ascontiguousarray(Wk[gsl, :].T).astype(bf),
            "wv": np.ascontiguousarray(Wv[gsl, :].T).astype(bf),
            "wo": np.ascontiguousarray(Wo[:, gsl].T).astype(bf),
            "cc": cc.astype(bf), "ss": ss.astype(bf),
            "cst": cst.astype(bf), "perm": permm.astype(bf),
            "onesr": onesr.astype(bf),
        })
    return in_maps


def kernel(x, cos, sin, Wq, Wk, Wv, Wo):
    from concourse.bass_utils import run_bass_kernel_spmd

    x = np.asarray(x, dtype=np.float32)
    cos = np.asarray(cos, dtype=np.float32)
    sin = np.asarray(sin, dtype=np.float32)
    Wq = np.asarray(Wq, dtype=np.float32)
    Wk = np.asarray(Wk, dtype=np.float32)
    Wv = np.asarray(Wv, dtype=np.float32)
    Wo = np.asarray(Wo, dtype=np.float32)

    nc = _get_nc()
    in_maps = make_in_maps(x, cos, sin, Wq, Wk, Wv, Wo)
    res = run_bass_kernel_spmd(nc, in_maps, core_ids=list(range(8)))
    outs = [np.asarray(r_["out"], dtype=np.float32) for r_ in res.results]
    return np.stack([outs[2 * b] + outs[2 * b + 1] for b in range(_B)], axis=0)


# revision 32
# speedup vs baseline: 1.0902x; 1.0103x over previous
"""Trainium2 Bass kernel for CausalSelfAttention (B=4, T=2048, C=768, H=6, D=128)
with RoPE + QK-RMSNorm.

Sharding: 8 cores = batch(4) x head-group(2, 3 heads each). Host sums the two
head-group c_proj partials per batch.

v3 design:
  - bf16 data plane (inputs, K/Q/V/A tiles): halves input DMA, enables DVE
    2x elementwise modes, removes the f32r 4-cycle penalty on narrow matmuls.
  - K's RMS-norm is folded into the exp's per-partition scale AP (never
    applied to the K tile).
  - softmax denominator comes free from the AV matmul: AV is computed
    transposed (out [q,129]) with a ones column appended to V, so column 128
    accumulates sum_k(A); normalized z^T transposes back via PE transposes.
  - phase A computes ALL projections + RoPE + norms (Q tiles persistent), so
    phase B's Activation engine runs Exp only -- no act-table reloads.
  - phase B interleaves qt's AV/tails/c_proj with qt+1's S/exp stream.
"""

import numpy as np

_B, _T, _C, _H, _D = 4, 2048, 768, 6, 128
_HPG = 3            # heads per group
_HD = _HPG * _D     # 384, per-group head dims
_NT = 4             # T tiles of 512
_TW = 512           # tile width (T_q)
_NKC = _T // 128    # 16 k-chunks of 128
_NCB = _C // 128    # 6 c_in chunks
_EPS = 1e-15

_cached = {}


def _build_nc():
    from contextlib import ExitStack
    from concourse import bacc, tile, mybir

    f32 = mybir.dt.float32
    f32r = mybir.dt.float32r
    bf16 = mybir.dt.bfloat16
    Act = mybir.ActivationFunctionType

    nc = bacc.Bacc("TRN2", target_bir_lowering=False, debug=False)
    _cached.setdefault("sections", []).clear()

    def _mark(label):
        _cached["sections"].append((label, nc.get_next_instruction_name()))

    xT = nc.dram_tensor("xT", (_C, _T), bf16, kind="ExternalInput").ap()
    wq = nc.dram_tensor("wq", (_C, _HD), bf16, kind="ExternalInput").ap()
    wk = nc.dram_tensor("wk", (_C, _HD), bf16, kind="ExternalInput").ap()
    wv = nc.dram_tensor("wv", (_C, _HD), bf16, kind="ExternalInput").ap()
    wo = nc.dram_tensor("wo", (_HD, _C), bf16, kind="ExternalInput").ap()
    cc = nc.dram_tensor("cc", (128, _T), bf16, kind="ExternalInput").ap()
    ss = nc.dram_tensor("ss", (128, _T), bf16, kind="ExternalInput").ap()
    # cst: [tri(128) | ident(128) | ones_col(1)] in bf16
    cst = nc.dram_tensor("cst", (128, 257), bf16, kind="ExternalInput").ap()
    onesr = nc.dram_tensor("onesr", (1, 128), bf16, kind="ExternalInput").ap()
    out = nc.dram_tensor("out", (_T, _C), bf16, kind="ExternalOutput").ap()

    with tile.TileContext(nc) as tc, ExitStack() as ctx, \
            nc.allow_low_precision(reason="bf16 data plane; psum accumulate f32"):
        # --- pools ---
        pc = ctx.enter_context(tc.tile_pool(name="pc", bufs=1))
        pg = ctx.enter_context(tc.tile_pool(name="pg", bufs=3))        # scratch
        pa = ctx.enter_context(tc.tile_pool(name="pa", bufs=1))        # A tiles
        psm = ctx.enter_context(tc.tile_pool(name="psm", bufs=3))      # small rows
        # psum pools (8 banks total)
        pps = ctx.enter_context(tc.tile_pool(name="pps", bufs=3, space="PSUM"))
        ppo = ctx.enter_context(tc.tile_pool(name="ppo", bufs=2, space="PSUM"))
        ppq = ctx.enter_context(tc.tile_pool(name="ppq", bufs=2, space="PSUM"))
        ppd = ctx.enter_context(tc.tile_pool(name="ppd", bufs=1, space="PSUM"))

        # --- inputs resident in SBUF; packed tiles, few large DMAs ---
        t_wka = pc.tile([128, _NCB * _HD], bf16, tag="wka", name="t_wka")
        nc.sync.dma_start(t_wka[:].rearrange("p (c h) -> p c h", c=_NCB),
                          wk.rearrange("(c p) h -> p c h", c=_NCB))
        t_xta = pc.tile([128, _NCB * _T], bf16, tag="xta", name="t_xta")
        xta_v = t_xta[:].rearrange("p (c t) -> p c t", c=_NCB)
        xT_v = xT.rearrange("(c p) t -> p c t", c=_NCB)
        nc.sync.dma_start(xta_v[:, :, 0:_TW], xT_v[:, :, 0:_TW])
        nc.sync.dma_start(xta_v[:, :, _TW:_T], xT_v[:, :, _TW:_T])
        t_wva = pc.tile([128, _NCB * _HD], bf16, tag="wva", name="t_wva")
        nc.sync.dma_start(t_wva[:].rearrange("p (c h) -> p c h", c=_NCB),
                          wv.rearrange("(c p) h -> p c h", c=_NCB))
        t_wqa = pc.tile([128, _NCB * _HD], bf16, tag="wqa", name="t_wqa")
        nc.sync.dma_start(t_wqa[:].rearrange("p (c h) -> p c h", c=_NCB),
                          wq.rearrange("(c p) h -> p c h", c=_NCB))
        t_ccss = pc.tile([128, 2 * _T], bf16, tag="ccss", name="t_ccss")
        nc.sync.dma_start(t_ccss[:, 0:_T], cc[:])
        nc.sync.dma_start(t_ccss[:, _T:2 * _T], ss[:])
        t_cst = pc.tile([128, 257], bf16, tag="cst")
        t_onesr = pc.tile([1, 128], bf16, tag="onesr")
        nc.sync.dma_start(t_cst[:], cst[:])
        nc.sync.dma_start(t_onesr[:], onesr[:])
        t_woa = pc.tile([128, _HPG * _C], bf16, tag="woa", name="t_woa")
        nc.sync.dma_start(t_woa[:].rearrange("p (c o) -> p c o", c=_HPG),
                          wo.rearrange("(c p) o -> p c o", c=_HPG))

        def _xt(c, a, b):
            return t_xta[:, c * _T + a:c * _T + b]

        def _wk(c, hsl):
            return t_wka[:, c * _HD + hsl.start:c * _HD + hsl.stop]

        def _wv(c):
            return t_wva[:, c * _HD:(c + 1) * _HD]

        def _wq(c, hsl):
            return t_wqa[:, c * _HD + hsl.start:c * _HD + hsl.stop]

        def _wo(c, osl):
            return t_woa[:, c * _C + osl.start:c * _C + osl.stop]

        t_tri = t_cst[:, 0:128]
        t_ident = t_cst[:, 128:256]
        t_ones_col = t_cst[:, 256:257]

        t_epsA = pc.tile([128, 1], f32, tag="epsA")   # K: 128*eps
        nc.gpsimd.memset(t_epsA[:], 128.0 * _EPS)
        t_epsB = pc.tile([128, 1], f32, tag="epsB")   # Q: eps
        nc.gpsimd.memset(t_epsB[:], _EPS)

        # persistent K^T (rope'd, UN-normalized) per head; V blocks w/ ones col
        t_kn = [pc.tile([128, _T], bf16, tag=f"kn{h}", name=f"kn{h}")
                for h in range(_HPG)]
        t_vo = [pc.tile([128, 387], bf16, tag=f"vo{tb}", name=f"vo{tb}")
                for tb in range(_NKC)]
        # exp scale columns: c_k = 1/sqrt(ms_k + 128 eps); col h*16+kc
        t_ck = pc.tile([128, _HPG * _NKC], f32, tag="ck", name="t_ck")
        # all Q tiles (rope'd + normalized), persistent through phase B
        t_q = [[pc.tile([128, _TW], bf16, tag=f"q{qt}_{h}", name=f"q{qt}_{h}")
                for h in range(_HPG)] for qt in range(_NT)]
        # A tiles: 16 k-chunks x 3 heads, reused across qt
        t_a = [[pa.tile([128, _TW], bf16, tag=f"a{h}_{kc}", name=f"a{h}_{kc}")
                for kc in range(_NKC)] for h in range(_HPG)]
        t_zT = [pc.tile([128, _TW], bf16, tag=f"zT{h}", name=f"zT{h}")
                for h in range(_HPG)]
        t_z = [pc.tile([128, _TW], bf16, tag=f"z{h}", name=f"z{h}")
               for h in range(_HPG)]

        def rope(dst_ap, col0):
            """In-place RoPE on dst_ap (128, 512) bf16 sbuf tile slice.
            Half-swap via SBUF->SBUF DMA; cc-mul on Pool, rest on DVE (2x)."""
            t_sw = pg.tile([128, _TW], bf16, tag="sw", name="t_sw")
            nc.sync.dma_start(t_sw[0:64, :], dst_ap[64:128, :])
            nc.sync.dma_start(t_sw[64:128, :], dst_ap[0:64, :])
            nc.gpsimd.tensor_mul(dst_ap, dst_ap, t_ccss[:, col0:col0 + _TW])
            nc.vector.tensor_mul(t_sw[:], t_sw[:], t_ccss[:, _T + col0:_T + col0 + _TW])
            nc.vector.tensor_add(dst_ap, dst_ap, t_sw[:])

        # ============ Phase A: projections, RoPE, norms ============
        for i in range(_NT):
            isl = slice(i * _TW, (i + 1) * _TW)
            for h in range(_HPG):
                hsl = slice(h * 128, (h + 1) * 128)
                p_k = pps.tile([128, _TW], f32, tag="ps", name="p_k")
                for c in range(_NCB):
                    nc.tensor.matmul(p_k[:], _wk(c, hsl), _xt(c, i * _TW, (i + 1) * _TW),
                                     start=(c == 0), stop=(c == _NCB - 1))
                nc.vector.tensor_copy(t_kn[h][:, isl], p_k[:])
        def v_proj(tb0, tb1):
            _mark(f'vproj{tb0}')
            for tb in range(tb0, tb1):
                bsl = slice(tb * 128, (tb + 1) * 128)
                p_v = ppo.tile([128, _HD], f32, tag="po", name="p_v")
                for c in range(_NCB):
                    nc.tensor.matmul(p_v[:], _xt(c, tb * 128, (tb + 1) * 128), _wv(c),
                                     start=(c == 0), stop=(c == _NCB - 1))
                vo_v = t_vo[tb][:].rearrange("p (h d) -> p h d", h=_HPG)[:, :, 0:128]
                pv_v = p_v[:].rearrange("p (h d) -> p h d", h=_HPG)
                nc.vector.tensor_copy(vo_v, pv_v)
                nc.gpsimd.memset(t_vo[tb][:].rearrange("p (h d) -> p h d", h=_HPG)
                                 [:, :, 128:129], 1.0)
        v_proj(0, 8)
        # K rope + norm stats per T-tile i; c_k col layout: i*12 + h*4 + j.
        # rsqrt via exp(-0.5 ln(x)) -- Ln and Exp share an act table, so the
        # Activation engine never reloads tables mid-kernel.
        def k_phase(i):
            _mark(f'k_phase{i}')
            isl = slice(i * _TW, (i + 1) * _TW)
            msl = slice(i * 12, (i + 1) * 12)
            for h in range(_HPG):
                rope(t_kn[h][:, isl], i * _TW)
            p_msk = ppd.tile([128, 12], f32, tag="pd", name="p_msk")
            for h in range(_HPG):
                t_sq = pg.tile([128, _TW], bf16, tag="sq", name="t_sq")
                nc.vector.tensor_mul(t_sq[:], t_kn[h][:, isl], t_kn[h][:, isl])
                for j in range(4):
                    col = h * 4 + j
                    nc.tensor.matmul(p_msk[:, col:col + 1],
                                     t_sq[:, j * 128:(j + 1) * 128],
                                     t_ones_col, start=True, stop=True)
            t_lk = pg.tile([128, 12], f32, tag="lq", name="t_lk")
            nc.scalar.activation(t_lk[:], p_msk[:], Act.Ln,
                                 bias=t_epsA[:], scale=1.0)
            nc.scalar.activation(t_ck[:, msl], t_lk[:], Act.Exp, scale=-0.5)

        def q_phase(qt):
            _mark(f'q_phase{qt}')
            """Q projections + rope + RMS-norm for one T_q tile (3 heads)."""
            qsl = slice(qt * _TW, (qt + 1) * _TW)
            for h in range(_HPG):
                hsl = slice(h * 128, (h + 1) * 128)
                p_q = ppq.tile([128, _TW], f32, tag="pq", name="p_q")
                for c in range(_NCB):
                    nc.tensor.matmul(p_q[:], _wq(c, hsl), _xt(c, qt * _TW, (qt + 1) * _TW),
                                     start=(c == 0), stop=(c == _NCB - 1))
                nc.vector.tensor_copy(t_q[qt][h][:], p_q[:])
            for h in range(_HPG):
                rope(t_q[qt][h][:], qt * _TW)
            p_msq = ppd.tile([128, 12], f32, tag="pd", name="p_msq")
            for h in range(_HPG):
                t_sq = pg.tile([128, _TW], bf16, tag="sq", name="t_sq")
                nc.vector.tensor_mul(t_sq[:], t_q[qt][h][:], t_q[qt][h][:])
                for qb in range(4):
                    col = h * 4 + qb
                    nc.tensor.matmul(p_msq[:, col:col + 1],
                                     t_sq[:, qb * 128:(qb + 1) * 128],
                                     t_ones_col, start=True, stop=True)
            t_lq = pg.tile([128, 12], f32, tag="lq", name="t_lq")
            nc.scalar.activation(t_lq[:], p_msq[:], Act.Ln,
                                 bias=t_epsB[:], scale=1.0 / 128.0)
            t_cq12 = pg.tile([128, 12], bf16, tag="cq12", name="t_cq12",
                             bufs=2)
            nc.scalar.activation(t_cq12[:], t_lq[:], Act.Exp, scale=-0.5)
            for h in range(_HPG):
                p_rq = ppd.tile([1, _TW], bf16, tag="pd", name="p_rq",
                                padded_shape=[1, 2 * _TW])
                for qb in range(4):
                    col = h * 4 + qb
                    nc.tensor.transpose(p_rq[0:1, qb * 128:(qb + 1) * 128],
                                        t_cq12[:, col:col + 1], t_ident)
                t_rqr = psm.tile([1, _TW], bf16, tag="rqr", name="t_rqr")
                nc.vector.tensor_copy(t_rqr[:], p_rq[:])
                p_bc = ppd.tile([128, _TW], f32, tag="pd", name="p_bc")
                nc.tensor.matmul(p_bc[:], t_onesr[:], t_rqr[:],
                                 start=True, stop=True)
                nc.vector.tensor_mul(t_q[qt][h][:], t_q[qt][h][:], p_bc[:])

        # ============ Phase B: attention + c_proj, qt-pipelined ============
        def emit_s_exp(qt, fillers=()):
            _mark(f's_exp{qt}')
            fillers = list(fillers)
            nchunk = 4 * qt + 4
            for h in range(_HPG):
                for kc in range(nchunk):
                    roff = 0 if kc < 4 * qt else (kc - 4 * qt) * 128
                    nsl = slice(roff, _TW)
                    ksl = slice(kc * 128, (kc + 1) * 128)
                    p_s = pps.tile([128, _TW], f32, tag="ps", name="p_s")
                    nc.tensor.matmul(p_s[:, nsl], t_kn[h][:, ksl],
                                     t_q[qt][h][:, nsl], start=True, stop=True)
                    ckc = (kc // 4) * 12 + h * 4 + (kc % 4)
                    nc.scalar.activation(t_a[h][kc][:, nsl], p_s[:, nsl],
                                         Act.Exp,
                                         scale=t_ck[:, ckc:ckc + 1])
                    if kc >= 4 * qt:  # diagonal chunk: triangular mask
                        dsl = slice(roff, roff + 128)
                        nc.gpsimd.tensor_mul(t_a[h][kc][:, dsl],
                                             t_a[h][kc][:, dsl], t_tri)
                if fillers:
                    fillers.pop(0)()

        def emit_av(qt):
            _mark(f'av{qt}')
            for h in range(_HPG):
                vsl = slice(h * 129, (h + 1) * 129)
                for qb in range(4):
                    qbsl = slice(qb * 128, (qb + 1) * 128)
                    kmax = 4 * qt + qb
                    p_ot = ppo.tile([128, 129], f32, tag="po", name="p_ot")
                    for kc in range(kmax + 1):
                        nc.tensor.matmul(p_ot[:], t_a[h][kc][:, qbsl],
                                         t_vo[kc][:, vsl],
                                         start=(kc == 0), stop=(kc == kmax))
                    t_rd = psm.tile([128, 1], f32, tag="rd", name="t_rd")
                    nc.vector.reciprocal(t_rd[:], p_ot[:, 128:129])
                    nc.vector.tensor_scalar_mul(t_zT[h][:, qbsl],
                                                p_ot[:, 0:128], t_rd[:])
                p_z2 = pps.tile([128, _TW], bf16, tag="ps", name="p_z2",
                                padded_shape=[128, 2 * _TW])
                for qb in range(4):
                    qbsl = slice(qb * 128, (qb + 1) * 128)
                    nc.tensor.transpose(p_z2[:, qbsl], t_zT[h][:, qbsl],
                                        t_ident)
                nc.scalar.copy(t_z[h][:], p_z2[:])

        def emit_cproj_tb(qt, tbs):
            _mark(f'cproj{qt}')
            for tb in tbs:
                bsl = slice(tb * 128, (tb + 1) * 128)
                rsl = slice(qt * _TW + tb * 128, qt * _TW + (tb + 1) * 128)
                t_ob = pg.tile([128, _C], bf16, tag="ob", name="t_ob", bufs=2)
                for nh in range(2):
                    osl = slice(nh * 384, (nh + 1) * 384)
                    p_c = ppq.tile([128, 384], f32, tag="pq", name="p_c")
                    for c in range(_HPG):
                        nc.tensor.matmul(p_c[:], t_z[c][:, bsl],
                                         _wo(c, osl),
                                         start=(c == 0), stop=(c == _HPG - 1))
                    nc.vector.tensor_copy(t_ob[:, osl], p_c[:])
                nc.sync.dma_start(out[rsl, :], t_ob[:])

        k_phase(0)
        q_phase(0)
        emit_s_exp(0)
        q_phase(1)
        for qt in range(_NT):
            if qt == 1:
                v_proj(8, 12)
            elif qt == 2:
                v_proj(12, 16)
            emit_av(qt)
            if qt + 1 < _NT:
                k_phase(qt + 1)
                emit_s_exp(qt + 1, fillers=[
                    lambda: emit_cproj_tb(qt, (0, 1)),
                    lambda: emit_cproj_tb(qt, (2, 3)),
                ])
            else:
                emit_cproj_tb(qt, (0, 1, 2, 3))
            if qt + 2 < _NT:
                q_phase(qt + 2)

    nc.compile()
    return nc


def _get_nc():
    if "nc" not in _cached:
        _cached["nc"] = _build_nc()
    return _cached["nc"]


def make_in_maps(x, cos, sin, Wq, Wk, Wv, Wo):
    import ml_dtypes
    bf = ml_dtypes.bfloat16

    cosT = np.ascontiguousarray(cos.reshape(_T, _D // 2).T)  # (64, T)
    sinT = np.ascontiguousarray(sin.reshape(_T, _D // 2).T)
    cc = np.concatenate([cosT, cosT], axis=0)                # (128, T)
    ss = np.concatenate([sinT, -sinT], axis=0)
    tri = (np.arange(128)[None, :] >= np.arange(128)[:, None]).astype(np.float32)
    ident = np.eye(128, dtype=np.float32)
    cst = np.concatenate([tri, ident, np.ones((128, 1), np.float32)], axis=1)
    onesr = np.ones((1, 128), dtype=np.float32)
    in_maps = []
    for core in range(8):
        b, g = divmod(core, 2)
        gsl = slice(g * _HD, (g + 1) * _HD)
        in_maps.append({
            "xT": np.ascontiguousarray(x[b].T).astype(bf),
            "wq": np.ascontiguousarray(Wq[gsl, :].T).astype(bf),
            "wk": np.ascontiguousarray(Wk[gsl, :].T).astype(bf),
            "wv": np.ascontiguousarray(Wv[gsl, :].T).astype(bf),
            "wo": np.ascontiguousarray(Wo[:, gsl].T).astype(bf),
            "cc": cc.astype(bf), "ss": ss.astype(bf),
            "cst": cst.astype(bf), "onesr": onesr.astype(bf),
        })
    return in_maps


def kernel(x, cos, sin, Wq, Wk, Wv, Wo):
    from concourse.bass_utils import run_bass_kernel_spmd

    x = np.asarray(x, dtype=np.float32)
    cos = np.asarray(cos, dtype=np.float32)
    sin = np.asarray(sin, dtype=np.float32)
    Wq = np.asarray(Wq, dtype=np.float32)
    Wk = np.asarray(Wk, dtype=np.float32)
    Wv = np.asarray(Wv, dtype=np.float32)
    Wo = np.asarray(Wo, dtype=np.float32)

    nc = _get_nc()
    in_maps = make_in_maps(x, cos, sin, Wq, Wk, Wv, Wo)
    res = run_bass_kernel_spmd(nc, in_maps, core_ids=list(range(8)))
    outs = [np.asarray(r_["out"], dtype=np.float32) for r_ in res.results]
    return np.stack([outs[2 * b] + outs[2 * b + 1] for b in range(_B)], axis=0)


# revision 33
# speedup vs baseline: 1.1009x; 1.0098x over previous
"""Trainium2 Bass kernel for CausalSelfAttention (B=4, T=2048, C=768, H=6, D=128)
with RoPE + QK-RMSNorm.

Sharding: 8 cores = batch(4) x head-group(2, 3 heads each). Host sums the two
head-group c_proj partials per batch.

v3 design:
  - bf16 data plane (inputs, K/Q/V/A tiles): halves input DMA, enables DVE
    2x elementwise modes, removes the f32r 4-cycle penalty on narrow matmuls.
  - K's RMS-norm is folded into the exp's per-partition scale AP (never
    applied to the K tile).
  - softmax denominator comes free from the AV matmul: AV is computed
    transposed (out [q,129]) with a ones column appended to V, so column 128
    accumulates sum_k(A); normalized z^T transposes back via PE transposes.
  - phase A computes ALL projections + RoPE + norms (Q tiles persistent), so
    phase B's Activation engine runs Exp only -- no act-table reloads.
  - phase B interleaves qt's AV/tails/c_proj with qt+1's S/exp stream.
"""

import numpy as np

_B, _T, _C, _H, _D = 4, 2048, 768, 6, 128
_HPG = 3            # heads per group
_HD = _HPG * _D     # 384, per-group head dims
_NT = 4             # T tiles of 512
_TW = 512           # tile width (T_q)
_NKC = _T // 128    # 16 k-chunks of 128
_NCB = _C // 128    # 6 c_in chunks
_EPS = 1e-15

_cached = {}


def _build_nc():
    from contextlib import ExitStack
    from concourse import bacc, tile, mybir

    f32 = mybir.dt.float32
    f32r = mybir.dt.float32r
    bf16 = mybir.dt.bfloat16
    Act = mybir.ActivationFunctionType

    nc = bacc.Bacc("TRN2", target_bir_lowering=False, debug=False)
    _cached.setdefault("sections", []).clear()

    def _mark(label):
        _cached["sections"].append((label, nc.get_next_instruction_name()))

    xT = nc.dram_tensor("xT", (_C, _T), bf16, kind="ExternalInput").ap()
    wq = nc.dram_tensor("wq", (_C, _HD), bf16, kind="ExternalInput").ap()
    wk = nc.dram_tensor("wk", (_C, _HD), bf16, kind="ExternalInput").ap()
    wv = nc.dram_tensor("wv", (_C, _HD), bf16, kind="ExternalInput").ap()
    wo = nc.dram_tensor("wo", (_HD, _C), bf16, kind="ExternalInput").ap()
    cc = nc.dram_tensor("cc", (128, _T), bf16, kind="ExternalInput").ap()
    ss = nc.dram_tensor("ss", (128, _T), bf16, kind="ExternalInput").ap()
    # cst: [tri(128) | ident(128) | ones_col(1)] in bf16
    cst = nc.dram_tensor("cst", (128, 257), bf16, kind="ExternalInput").ap()
    onesr = nc.dram_tensor("onesr", (1, 128), bf16, kind="ExternalInput").ap()
    out = nc.dram_tensor("out", (_T, _C), bf16, kind="ExternalOutput").ap()

    with tile.TileContext(nc) as tc, ExitStack() as ctx, \
            nc.allow_low_precision(reason="bf16 data plane; psum accumulate f32"):
        # --- pools ---
        pc = ctx.enter_context(tc.tile_pool(name="pc", bufs=1))
        pg = ctx.enter_context(tc.tile_pool(name="pg", bufs=3))        # scratch
        pa = ctx.enter_context(tc.tile_pool(name="pa", bufs=1))        # A tiles
        psm = ctx.enter_context(tc.tile_pool(name="psm", bufs=3))      # small rows
        # psum pools (8 banks total)
        pps = ctx.enter_context(tc.tile_pool(name="pps", bufs=2, space="PSUM"))
        ppo = ctx.enter_context(tc.tile_pool(name="ppo", bufs=3, space="PSUM"))
        ppq = ctx.enter_context(tc.tile_pool(name="ppq", bufs=2, space="PSUM"))
        ppd = ctx.enter_context(tc.tile_pool(name="ppd", bufs=1, space="PSUM"))

        # --- inputs resident in SBUF; packed tiles, few large DMAs ---
        t_wka = pc.tile([128, _NCB * _HD], bf16, tag="wka", name="t_wka")
        nc.sync.dma_start(t_wka[:].rearrange("p (c h) -> p c h", c=_NCB),
                          wk.rearrange("(c p) h -> p c h", c=_NCB))
        t_xta = pc.tile([128, _NCB * _T], bf16, tag="xta", name="t_xta")
        xta_v = t_xta[:].rearrange("p (c t) -> p c t", c=_NCB)
        xT_v = xT.rearrange("(c p) t -> p c t", c=_NCB)
        nc.sync.dma_start(xta_v[:, :, 0:_TW], xT_v[:, :, 0:_TW])
        nc.sync.dma_start(xta_v[:, :, _TW:_T], xT_v[:, :, _TW:_T])
        t_wva = pc.tile([128, _NCB * _HD], bf16, tag="wva", name="t_wva")
        nc.sync.dma_start(t_wva[:].rearrange("p (c h) -> p c h", c=_NCB),
                          wv.rearrange("(c p) h -> p c h", c=_NCB))
        t_wqa = pc.tile([128, _NCB * _HD], bf16, tag="wqa", name="t_wqa")
        nc.sync.dma_start(t_wqa[:].rearrange("p (c h) -> p c h", c=_NCB),
                          wq.rearrange("(c p) h -> p c h", c=_NCB))
        t_ccss = pc.tile([128, 2 * _T], bf16, tag="ccss", name="t_ccss")
        nc.sync.dma_start(t_ccss[:, 0:_T], cc[:])
        nc.sync.dma_start(t_ccss[:, _T:2 * _T], ss[:])
        t_cst = pc.tile([128, 257], bf16, tag="cst")
        t_onesr = pc.tile([1, 128], bf16, tag="onesr")
        nc.sync.dma_start(t_cst[:], cst[:])
        nc.sync.dma_start(t_onesr[:], onesr[:])
        t_woa = pc.tile([128, _HPG * _C], bf16, tag="woa", name="t_woa")
        nc.sync.dma_start(t_woa[:].rearrange("p (c o) -> p c o", c=_HPG),
                          wo.rearrange("(c p) o -> p c o", c=_HPG))

        def _xt(c, a, b):
            return t_xta[:, c * _T + a:c * _T + b]

        def _wk(c, hsl):
            return t_wka[:, c * _HD + hsl.start:c * _HD + hsl.stop]

        def _wv(c):
            return t_wva[:, c * _HD:(c + 1) * _HD]

        def _wq(c, hsl):
            return t_wqa[:, c * _HD + hsl.start:c * _HD + hsl.stop]

        def _wo(c, osl):
            return t_woa[:, c * _C + osl.start:c * _C + osl.stop]

        t_tri = t_cst[:, 0:128]
        t_ident = t_cst[:, 128:256]
        t_ones_col = t_cst[:, 256:257]

        t_epsA = pc.tile([128, 1], f32, tag="epsA")   # K: 128*eps
        nc.gpsimd.memset(t_epsA[:], 128.0 * _EPS)
        t_epsB = pc.tile([128, 1], f32, tag="epsB")   # Q: eps
        nc.gpsimd.memset(t_epsB[:], _EPS)

        # persistent K^T (rope'd, UN-normalized) per head; V blocks w/ ones col
        t_kn = [pc.tile([128, _T], bf16, tag=f"kn{h}", name=f"kn{h}")
                for h in range(_HPG)]
        t_vo = [pc.tile([128, 387], bf16, tag=f"vo{tb}", name=f"vo{tb}")
                for tb in range(_NKC)]
        # exp scale columns: c_k = 1/sqrt(ms_k + 128 eps); col h*16+kc
        t_ck = pc.tile([128, _HPG * _NKC], f32, tag="ck", name="t_ck")
        # all Q tiles (rope'd + normalized), persistent through phase B
        t_q = [[pc.tile([128, _TW], bf16, tag=f"q{qt}_{h}", name=f"q{qt}_{h}")
                for h in range(_HPG)] for qt in range(_NT)]
        # A tiles: 16 k-chunks x 3 heads, reused across qt
        t_a = [[pa.tile([128, _TW], bf16, tag=f"a{h}_{kc}", name=f"a{h}_{kc}")
                for kc in range(_NKC)] for h in range(_HPG)]
        t_zT = [pc.tile([128, _TW], bf16, tag=f"zT{h}", name=f"zT{h}")
                for h in range(_HPG)]
        t_z = [pc.tile([128, _TW], bf16, tag=f"z{h}", name=f"z{h}")
               for h in range(_HPG)]

        def rope(dst_ap, col0):
            """In-place RoPE on dst_ap (128, 512) bf16 sbuf tile slice.
            Half-swap via SBUF->SBUF DMA; cc-mul on Pool, rest on DVE (2x)."""
            t_sw = pg.tile([128, _TW], bf16, tag="sw", name="t_sw")
            nc.sync.dma_start(t_sw[0:64, :], dst_ap[64:128, :])
            nc.sync.dma_start(t_sw[64:128, :], dst_ap[0:64, :])
            nc.gpsimd.tensor_mul(dst_ap, dst_ap, t_ccss[:, col0:col0 + _TW])
            nc.vector.tensor_mul(t_sw[:], t_sw[:], t_ccss[:, _T + col0:_T + col0 + _TW])
            nc.vector.tensor_add(dst_ap, dst_ap, t_sw[:])

        # ============ Phase A: projections, RoPE, norms ============
        for i in range(_NT):
            isl = slice(i * _TW, (i + 1) * _TW)
            for h in range(_HPG):
                hsl = slice(h * 128, (h + 1) * 128)
                p_k = pps.tile([128, _TW], f32, tag="ps", name="p_k")
                for c in range(_NCB):
                    nc.tensor.matmul(p_k[:], _wk(c, hsl), _xt(c, i * _TW, (i + 1) * _TW),
                                     start=(c == 0), stop=(c == _NCB - 1))
                nc.vector.tensor_copy(t_kn[h][:, isl], p_k[:])
        def v_proj(tb0, tb1):
            _mark(f'vproj{tb0}')
            for tb in range(tb0, tb1):
                bsl = slice(tb * 128, (tb + 1) * 128)
                p_v = ppo.tile([128, _HD], f32, tag="po", name="p_v")
                for c in range(_NCB):
                    nc.tensor.matmul(p_v[:], _xt(c, tb * 128, (tb + 1) * 128), _wv(c),
                                     start=(c == 0), stop=(c == _NCB - 1))
                vo_v = t_vo[tb][:].rearrange("p (h d) -> p h d", h=_HPG)[:, :, 0:128]
                pv_v = p_v[:].rearrange("p (h d) -> p h d", h=_HPG)
                nc.vector.tensor_copy(vo_v, pv_v)
                nc.gpsimd.memset(t_vo[tb][:].rearrange("p (h d) -> p h d", h=_HPG)
                                 [:, :, 128:129], 1.0)
        v_proj(0, 8)
        # K rope + norm stats per T-tile i; c_k col layout: i*12 + h*4 + j.
        # rsqrt via exp(-0.5 ln(x)) -- Ln and Exp share an act table, so the
        # Activation engine never reloads tables mid-kernel.
        def k_phase(i):
            _mark(f'k_phase{i}')
            isl = slice(i * _TW, (i + 1) * _TW)
            msl = slice(i * 12, (i + 1) * 12)
            for h in range(_HPG):
                rope(t_kn[h][:, isl], i * _TW)
            p_msk = ppd.tile([128, 12], f32, tag="pd", name="p_msk")
            for h in range(_HPG):
                t_sq = pg.tile([128, _TW], bf16, tag="sq", name="t_sq")
                nc.vector.tensor_mul(t_sq[:], t_kn[h][:, isl], t_kn[h][:, isl])
                for j in range(4):
                    col = h * 4 + j
                    nc.tensor.matmul(p_msk[:, col:col + 1],
                                     t_sq[:, j * 128:(j + 1) * 128],
                                     t_ones_col, start=True, stop=True)
            t_lk = pg.tile([128, 12], f32, tag="lq", name="t_lk")
            nc.scalar.activation(t_lk[:], p_msk[:], Act.Ln,
                                 bias=t_epsA[:], scale=1.0)
            nc.scalar.activation(t_ck[:, msl], t_lk[:], Act.Exp, scale=-0.5)

        def q_phase(qt):
            _mark(f'q_phase{qt}')
            """Q projections + rope + RMS-norm for one T_q tile (3 heads)."""
            qsl = slice(qt * _TW, (qt + 1) * _TW)
            for h in range(_HPG):
                hsl = slice(h * 128, (h + 1) * 128)
                p_q = ppq.tile([128, _TW], f32, tag="pq", name="p_q")
                for c in range(_NCB):
                    nc.tensor.matmul(p_q[:], _wq(c, hsl), _xt(c, qt * _TW, (qt + 1) * _TW),
                                     start=(c == 0), stop=(c == _NCB - 1))
                nc.vector.tensor_copy(t_q[qt][h][:], p_q[:])
            for h in range(_HPG):
                rope(t_q[qt][h][:], qt * _TW)
            p_msq = ppd.tile([128, 12], f32, tag="pd", name="p_msq")
            for h in range(_HPG):
                t_sq = pg.tile([128, _TW], bf16, tag="sq", name="t_sq")
                nc.vector.tensor_mul(t_sq[:], t_q[qt][h][:], t_q[qt][h][:])
                for qb in range(4):
                    col = h * 4 + qb
                    nc.tensor.matmul(p_msq[:, col:col + 1],
                                     t_sq[:, qb * 128:(qb + 1) * 128],
                                     t_ones_col, start=True, stop=True)
            t_lq = pg.tile([128, 12], f32, tag="lq", name="t_lq")
            nc.scalar.activation(t_lq[:], p_msq[:], Act.Ln,
                                 bias=t_epsB[:], scale=1.0 / 128.0)
            t_cq12 = pg.tile([128, 12], bf16, tag="cq12", name="t_cq12",
                             bufs=2)
            nc.scalar.activation(t_cq12[:], t_lq[:], Act.Exp, scale=-0.5)
            for h in range(_HPG):
                p_rq = ppd.tile([1, _TW], bf16, tag="pd", name="p_rq",
                                padded_shape=[1, 2 * _TW])
                for qb in range(4):
                    col = h * 4 + qb
                    nc.tensor.transpose(p_rq[0:1, qb * 128:(qb + 1) * 128],
                                        t_cq12[:, col:col + 1], t_ident)
                t_rqr = psm.tile([1, _TW], bf16, tag="rqr", name="t_rqr")
                nc.vector.tensor_copy(t_rqr[:], p_rq[:])
                p_bc = ppd.tile([128, _TW], f32, tag="pd", name="p_bc")
                nc.tensor.matmul(p_bc[:], t_onesr[:], t_rqr[:],
                                 start=True, stop=True)
                nc.vector.tensor_mul(t_q[qt][h][:], t_q[qt][h][:], p_bc[:])

        # ============ Phase B: attention + c_proj, qt-pipelined ============
        def emit_s_exp(qt, fillers=()):
            _mark(f's_exp{qt}')
            fillers = list(fillers)
            nchunk = 4 * qt + 4
            for h in range(_HPG):
                for kc in range(nchunk):
                    roff = 0 if kc < 4 * qt else (kc - 4 * qt) * 128
                    nsl = slice(roff, _TW)
                    ksl = slice(kc * 128, (kc + 1) * 128)
                    p_s = pps.tile([128, _TW], f32, tag="ps", name="p_s")
                    nc.tensor.matmul(p_s[:, nsl], t_kn[h][:, ksl],
                                     t_q[qt][h][:, nsl], start=True, stop=True)
                    ckc = (kc // 4) * 12 + h * 4 + (kc % 4)
                    nc.scalar.activation(t_a[h][kc][:, nsl], p_s[:, nsl],
                                         Act.Exp,
                                         scale=t_ck[:, ckc:ckc + 1])
                    if kc >= 4 * qt:  # diagonal chunk: triangular mask
                        dsl = slice(roff, roff + 128)
                        nc.gpsimd.tensor_mul(t_a[h][kc][:, dsl],
                                             t_a[h][kc][:, dsl], t_tri)
                if fillers:
                    fillers.pop(0)()

        def emit_av(qt):
            _mark(f'av{qt}')
            for h in range(_HPG):
                vsl = slice(h * 129, (h + 1) * 129)
                for qb in range(4):
                    qbsl = slice(qb * 128, (qb + 1) * 128)
                    kmax = 4 * qt + qb
                    p_ot = ppo.tile([128, 129], f32, tag="po", name="p_ot")
                    for kc in range(kmax + 1):
                        nc.tensor.matmul(p_ot[:], t_a[h][kc][:, qbsl],
                                         t_vo[kc][:, vsl],
                                         start=(kc == 0), stop=(kc == kmax))
                    t_rd = psm.tile([128, 1], f32, tag="rd", name="t_rd")
                    nc.vector.reciprocal(t_rd[:], p_ot[:, 128:129])
                    nc.vector.tensor_scalar_mul(t_zT[h][:, qbsl],
                                                p_ot[:, 0:128], t_rd[:])
                p_z2 = pps.tile([128, _TW], bf16, tag="ps", name="p_z2",
                                padded_shape=[128, 2 * _TW])
                for qb in range(4):
                    qbsl = slice(qb * 128, (qb + 1) * 128)
                    nc.tensor.transpose(p_z2[:, qbsl], t_zT[h][:, qbsl],
                                        t_ident)
                nc.scalar.copy(t_z[h][:], p_z2[:])

        def emit_cproj_tb(qt, tbs):
            _mark(f'cproj{qt}')
            for tb in tbs:
                bsl = slice(tb * 128, (tb + 1) * 128)
                rsl = slice(qt * _TW + tb * 128, qt * _TW + (tb + 1) * 128)
                t_ob = pg.tile([128, _C], bf16, tag="ob", name="t_ob", bufs=2)
                for nh in range(2):
                    osl = slice(nh * 384, (nh + 1) * 384)
                    p_c = ppq.tile([128, 384], f32, tag="pq", name="p_c")
                    for c in range(_HPG):
                        nc.tensor.matmul(p_c[:], t_z[c][:, bsl],
                                         _wo(c, osl),
                                         start=(c == 0), stop=(c == _HPG - 1))
                    nc.vector.tensor_copy(t_ob[:, osl], p_c[:])
                nc.sync.dma_start(out[rsl, :], t_ob[:])

        k_phase(0)
        q_phase(0)
        emit_s_exp(0)
        q_phase(1)
        for qt in range(_NT):
            if qt == 1:
                v_proj(8, 12)
            elif qt == 2:
                v_proj(12, 16)
            emit_av(qt)
            if qt + 1 < _NT:
                k_phase(qt + 1)
                emit_s_exp(qt + 1, fillers=[
                    lambda: emit_cproj_tb(qt, (0, 1)),
                    lambda: emit_cproj_tb(qt, (2, 3)),
                ])
            else:
                emit_cproj_tb(qt, (0, 1, 2, 3))
            if qt + 2 < _NT:
                q_phase(qt + 2)

    nc.compile()
    return nc


def _get_nc():
    if "nc" not in _cached:
        _cached["nc"] = _build_nc()
    return _cached["nc"]


def make_in_maps(x, cos, sin, Wq, Wk, Wv, Wo):
    import ml_dtypes
    bf = ml_dtypes.bfloat16

    cosT = np.ascontiguousarray(cos.reshape(_T, _D // 2).T)  # (64, T)
    sinT = np.ascontiguousarray(sin.reshape(_T, _D // 2).T)
    cc = np.concatenate([cosT, cosT], axis=0)                # (128, T)
    ss = np.concatenate([sinT, -sinT], axis=0)
    tri = (np.arange(128)[None, :] >= np.arange(128)[:, None]).astype(np.float32)
    ident = np.eye(128, dtype=np.float32)
    cst = np.concatenate([tri, ident, np.ones((128, 1), np.float32)], axis=1)
    onesr = np.ones((1, 128), dtype=np.float32)
    in_maps = []
    for core in range(8):
        b, g = divmod(core, 2)
        gsl = slice(g * _HD, (g + 1) * _HD)
        in_maps.append({
            "xT": np.ascontiguousarray(x[b].T).astype(bf),
            "wq": np.ascontiguousarray(Wq[gsl, :].T).astype(bf),
            "wk": np.ascontiguousarray(Wk[gsl, :].T).astype(bf),
            "wv": np.ascontiguousarray(Wv[gsl, :].T).astype(bf),
            "wo": np.ascontiguousarray(Wo[:, gsl].T).astype(bf),
            "cc": cc.astype(bf), "ss": ss.astype(bf),
            "cst": cst.astype(bf), "onesr": onesr.astype(bf),
        })
    return in_maps


def kernel(x, cos, sin, Wq, Wk, Wv, Wo):
    from concourse.bass_utils import run_bass_kernel_spmd

    x = np.asarray(x, dtype=np.float32)
    cos = np.asarray(cos, dtype=np.float32)
    sin = np.asarray(sin, dtype=np.float32)
    Wq = np.asarray(Wq, dtype=np.float32)
    Wk = np.asarray(Wk, dtype=np.float32)
    Wv = np.asarray(Wv, dtype=np.float32)
    Wo = np.asarray(Wo, dtype=np.float32)

    nc = _get_nc()
    in_maps = make_in_maps(x, cos, sin, Wq, Wk, Wv, Wo)
    res = run_bass_kernel_spmd(nc, in_maps, core_ids=list(range(8)))
    outs = [np.asarray(r_["out"], dtype=np.float32) for r_ in res.results]
    return np.stack([outs[2 * b] + outs[2 * b + 1] for b in range(_B)], axis=0)
